# revision 1
# baseline (speedup 1.0000x reference)
"""GCN link-predictor kernel for 8 Trainium2 NeuronCores (Bass/Tile).

Strategy (SPMD, single program on 8 cores, no core-dependent addressing):
  - Host: append self loops, compute deg/dinv and per-edge norm =
    dinv[src]*ew*dinv[dst] (cheap O(E) scalar prep, same flavor as the
    sort/bucket/pad already done host-side).  Partition nodes into 8
    contiguous ranges (12500/core, padded to 12544 = 98 tiles of 128).
    Core q owns all edges whose dst lies in its range, grouped per
    128-node destination tile, then per source BANK (4 equal banks of
    the padded node table, <32768 rows each so gather indices fit int16),
    padded to uniform chunks of 128 edges.
  - layer GEMM: h = x @ W over the FULL node table on every core
    (replicated compute beats an extra collective); loads/stores batched
    8 tiles (256KB) per DMA; psum banks hold 4 tiles -> 1 wide eviction.
  - aggregation: per group of 7 dst tiles, FOUR dma_gather instructions
    (one per source bank) fetch ALL h rows for the group's edges.
    dma_gather (InstDMAGatherAnt, gpsimd mlp library) moves num_idxs
    256B rows per instruction, so the ~1us SWDGE fixed cost is amortized
    over ~4000 rows instead of 128 (the old per-chunk indirect-DMA paid
    it per 128 rows = ~5ms of serialized Pool time).  One-hot W built on
    DVE from iota/meta; K matmuls accumulate in PSUM per dst tile.
    Layer 1 uses lhsT=h, rhs=W so psum comes out [feat, node] = already
    transposed for the layer-2 GEMM (no PE transpose), bias+relu fused
    in one scalar activation.  Layer 2 uses lhsT=W, rhs=h -> node-major
    for the label gathers.
  - AllGather o1t (feature-major) and o2 (node-major) between phases.
  - labels: host groups the label pairs by (bank(el0), bank(el1)) -> 16
    streams, so each stream's a-rows and b-rows each come from a single
    bank via one dma_gather; res slots are permuted back on the host.
    res = sum(a*b*w_vec) + sum(lin_b) where w_vec = lin_W @ 1.
"""

import os
import sys

import numpy as np

for _p in ("/opt/trn_rl_repo",):
    if _p not in sys.path:
        sys.path.insert(0, _p)

import ml_dtypes  # noqa: E402

import concourse.bacc as bacc  # noqa: E402
import concourse.bass as bass  # noqa: E402
import concourse.mybir as mybir  # noqa: E402
from concourse.bass_utils import run_bass_kernel_spmd  # noqa: E402
from concourse.library_config import mlp  # noqa: E402
from concourse.tile import TileContext  # noqa: E402

P = 128
NC = 8
NBANK = 4
BF = mybir.dt.bfloat16
F32 = mybir.dt.float32
I16 = mybir.dt.int16
I32 = mybir.dt.int32

LAST_EXEC_NS = None
LAST_RESULTS = None


class Cfg:
    def __init__(self, n_nodes, n_labels):
        assert n_nodes % NC == 0
        self.n_nodes = n_nodes
        self.nodes_per_core = n_nodes // NC
        self.tiles_per_core = -(-self.nodes_per_core // P)
        self.n_loc = self.tiles_per_core * P
        self.n_pad = NC * self.n_loc
        self.n_labels = n_labels
        self.lab_per_core = -(-n_labels // NC)
        self.lab_chunks = -(-self.lab_per_core // P)
        assert self.n_pad % (NBANK * P) == 0
        self.bank_rows = self.n_pad // NBANK
        assert self.bank_rows < (1 << 15)


FULL = Cfg(100000, 200000)

GEMM_G = 8   # node tiles per GEMM load/store group
AGG_G = 4    # dst tiles per aggregation gather group
# max 128-row chunks per dma_gather instruction (SWDGE descriptor-ring cap)
MAXCH = int(os.environ.get("KERNEL_MAXCH", "8"))
SCRATCH = int(os.environ.get("KERNEL_SCRATCH", "16384"))


# ---------------------------------------------------------------- host prep


def _pad_ids(cfg, ids):
    q, l = np.divmod(ids, cfg.nodes_per_core)
    q = np.minimum(q, NC - 1)
    l = ids - q * cfg.nodes_per_core
    return q * cfg.n_loc + l, q, l


def preprocess(cfg, x, edge_index, edge_weight, edge_label_index):
    n = cfg.n_nodes
    T = cfg.tiles_per_core
    BR = cfg.bank_rows
    src = np.concatenate([edge_index[0], np.arange(n)]).astype(np.int64)
    dst = np.concatenate([edge_index[1], np.arange(n)]).astype(np.int64)
    ew = np.concatenate(
        [edge_weight.astype(np.float32), np.ones(n, np.float32)]
    )
    # symmetric GCN normalization, computed host-side (scalar metadata prep)
    deg = np.bincount(dst, weights=ew, minlength=n).astype(np.float32)
    dinv = (1.0 / np.sqrt(np.maximum(deg, 1e-12))).astype(np.float32)
    norm = dinv[src] * ew * dinv[dst]

    src_pad, _, _ = _pad_ids(cfg, src)
    _, dq, dl = _pad_ids(cfg, dst)
    lt_e = dl // P
    b_e = src_pad // BR
    srcl_e = (src_pad - b_e * BR).astype(np.int16)
    dstl_e = (dl % P).astype(np.int64)

    key = (dq * T + lt_e) * NBANK + b_e
    counts = np.bincount(key, minlength=NC * T * NBANK).reshape(
        NC, T, NBANK
    )
    kb = -(-counts.max(axis=0) // P)  # [T, NBANK] chunks (0 allowed)
    Ksum = kb.sum(axis=1)  # [T]
    assert (Ksum >= 1).all()
    KSMAX = int(Ksum.max())
    kboff = np.zeros((T, NBANK), np.int64)
    kboff[:, 1:] = np.cumsum(kb, axis=1)[:, :-1]
    mc0 = np.zeros(T + 1, np.int64)
    mc0[1:] = np.cumsum(Ksum)
    C = int(mc0[-1])

    # group layout: per group of AGG_G tiles, the gather buffer holds the
    # bank-0 chunks of all its tiles, then bank-1, ...  hoff maps each
    # tile's meta-order chunk j to its buffer position.
    groups = [
        list(range(g0, min(g0 + AGG_G, T))) for g0 in range(0, T, AGG_G)
    ]
    seg = []      # per group: [NBANK+1] chunk offsets of bank streams
    hoffs = []    # per group: {lt: [buffer chunk pos per meta chunk]}
    gcol0 = []    # per group: column offset into gidx
    nchg = []     # per group: total chunks
    bstream = np.zeros((T, NBANK), np.int64)
    col = 0
    for tiles in groups:
        pos = 0
        segs = []
        hoff = {lt: [0] * int(Ksum[lt]) for lt in tiles}
        for b in range(NBANK):
            segs.append(pos)
            for lt in tiles:
                bstream[lt, b] = pos
                for k in range(int(kb[lt, b])):
                    hoff[lt][int(kboff[lt, b]) + k] = pos
                    pos += 1
        segs.append(pos)
        seg.append(segs)
        hoffs.append(hoff)
        gcol0.append(col)
        nchg.append(pos)
        col += pos * 8
    ICOLS = col
    NCHMAX = max(nchg)

    # place edges
    order = np.argsort(key, kind="stable")
    sk = key[order]
    starts = np.zeros(NC * T * NBANK + 1, np.int64)
    starts[1:] = np.cumsum(counts.reshape(-1))
    pos_in = np.arange(len(order)) - starts[sk]
    core_o = sk // (T * NBANK)
    lt_o = (sk // NBANK) % T
    b_o = sk % NBANK

    mcol = mc0[lt_o] + kboff[lt_o, b_o] + pos_in // P
    mpart = pos_in % P
    dstl_a = np.zeros((NC, P, C), np.float32)
    norm_a = np.zeros((NC, P, C), np.float32)
    dstl_a[core_o, mpart, mcol] = dstl_e[order]
    norm_a[core_o, mpart, mcol] = norm[order]
    meta = np.concatenate([dstl_a, norm_a], axis=-1).astype(
        ml_dtypes.bfloat16
    )

    gc_of_tile = np.array([gcol0[lt // AGG_G] for lt in range(T)])
    colbase = gc_of_tile[:, None] + bstream * 8  # [T, NBANK]
    gcol = colbase[lt_o, b_o] + pos_in // 16
    gpart = pos_in % 16
    gidx16 = np.zeros((NC, 16, ICOLS), np.int16)
    gidx16[core_o, gpart, gcol] = srcl_e[order]
    gidx = np.ascontiguousarray(np.tile(gidx16, (1, 8, 1)))

    # ---- labels, grouped per (bank(el0), bank(el1)) ----
    el_pad, _, _ = _pad_ids(cfg, edge_label_index.astype(np.int64))
    eb0 = el_pad[0] // BR
    el0l = (el_pad[0] - eb0 * BR).astype(np.int16)
    eb1 = el_pad[1] // BR
    el1l = (el_pad[1] - eb1 * BR).astype(np.int16)
    bp_all = eb0 * NBANK + eb1
    lpc = cfg.lab_per_core
    NBP = NBANK * NBANK
    cnts = np.zeros((NC, NBP), np.int64)
    for q in range(NC):
        lo, hi = q * lpc, min((q + 1) * lpc, cfg.n_labels)
        cnts[q] = np.bincount(bp_all[lo:hi], minlength=NBP)
    lkb = (-(-cnts.max(axis=0) // P)).astype(np.int64)  # [NBP]
    lchunk0 = np.zeros(NBP + 1, np.int64)
    lchunk0[1:] = np.cumsum(lkb)
    LCp = int(lchunk0[-1])
    LABMAX = int(lkb.max())
    # idx columns: per bp, [a stream | b stream]
    lcol0a = lchunk0[:-1] * 16
    lcol0b = lcol0a + lkb * 8
    LICOLS = LCp * 16

    lidx16 = np.zeros((NC, 16, LICOLS), np.int16)
    order_arr = np.full((NC, LCp * P), -1, np.int64)
    for q in range(NC):
        lo, hi = q * lpc, min((q + 1) * lpc, cfg.n_labels)
        bp_q = bp_all[lo:hi]
        oq = np.argsort(bp_q, kind="stable")
        sbp = bp_q[oq]
        st = np.zeros(NBP + 1, np.int64)
        st[1:] = np.cumsum(cnts[q])
        pos = np.arange(len(oq)) - st[sbp]
        cola = lcol0a[sbp] + pos // 16
        colb = lcol0b[sbp] + pos // 16
        prt = pos % 16
        lidx16[q, prt, cola] = el0l[lo:hi][oq]
        lidx16[q, prt, colb] = el1l[lo:hi][oq]
        slot = (lchunk0[sbp] + pos // P) * P + pos % P
        order_arr[q, slot] = lo + oq
    lidx = np.ascontiguousarray(np.tile(lidx16, (1, 8, 1)))

    # node features, padded + transposed
    pid_all, _, _ = _pad_ids(cfg, np.arange(n))
    x_pad = np.zeros((cfg.n_pad, P), np.float32)
    x_pad[pid_all] = x
    xT = np.ascontiguousarray(x_pad.T).astype(ml_dtypes.bfloat16)

    iota_rep = np.tile(
        np.arange(P, dtype=np.float32)[None, :], (P, KSMAX)
    ).astype(ml_dtypes.bfloat16)

    layout = dict(
        kb=kb, Ksum=[int(v) for v in Ksum], mc0=[int(v) for v in mc0],
        C=C, KSMAX=KSMAX, groups=groups, seg=seg, hoffs=hoffs,
        gcol0=gcol0, nchg=nchg, ICOLS=ICOLS, NCHMAX=NCHMAX,
        lkb=[int(v) for v in lkb], lchunk0=[int(v) for v in lchunk0],
        LCp=LCp, LABMAX=LABMAX,
        lcol0a=[int(v) for v in lcol0a], lcol0b=[int(v) for v in lcol0b],
        LICOLS=LICOLS,
    )
    return dict(gidx=gidx, meta=meta, lidx=lidx, xT=xT,
                order_arr=order_arr, iota_rep=iota_rep, layout=layout)


# ------------------------------------------------------------- bass program


def build_program(cfg, lay, linb_sum, phase=99):
    T = cfg.tiles_per_core
    NPAD, NLOC = cfg.n_pad, cfg.n_loc
    BR = cfg.bank_rows
    GT = NC * T
    rg = [list(range(NC))]
    C, KSMAX, NCHMAX = lay["C"], lay["KSMAX"], lay["NCHMAX"]
    Ksum, mc0, kb = lay["Ksum"], lay["mc0"], lay["kb"]
    groups, seg, hoffs = lay["groups"], lay["seg"], lay["hoffs"]
    gcol0, nchg = lay["gcol0"], lay["nchg"]
    LCp, LABMAX = lay["LCp"], lay["LABMAX"]
    lkb, lchunk0 = lay["lkb"], lay["lchunk0"]
    lcol0a, lcol0b = lay["lcol0a"], lay["lcol0b"]

    nc = bacc.Bacc(None, target_bir_lowering=False, debug=False,
                   dynamic_dma_scratch_size=SCRATCH, num_swdge_queues=4)
    qrr = [0]

    def next_q():
        qrr[0] = (qrr[0] + 1) % 4
        return qrr[0]

    xT = nc.declare_dram_parameter("xT", [P, NPAD], BF, False)
    gidx_d = nc.declare_dram_parameter("gidx", [P, lay["ICOLS"]], I16, False)
    meta_d = nc.declare_dram_parameter("meta", [P, 2 * C], BF, False)
    iota_d = nc.declare_dram_parameter("iota", [P, KSMAX * P], BF, False)
    lidx_d = nc.declare_dram_parameter("lidx", [P, lay["LICOLS"]], I16, False)
    w1_d = nc.declare_dram_parameter("w1", [P, P], BF, False)
    w2_d = nc.declare_dram_parameter("w2", [P, P], BF, False)
    b1c_d = nc.declare_dram_parameter("b1c", [P, 1], F32, False)
    b2_d = nc.declare_dram_parameter("b2bc", [P, P], F32, False)
    wv_d = nc.declare_dram_parameter("wvrep", [P, LABMAX * P], F32, False)
    res_d = nc.declare_dram_parameter("res", [P, LCp], F32, True)

    htab1 = nc.dram_tensor("htab1", [NPAD, P], BF)
    htab2 = nc.dram_tensor("htab2", [NPAD, P], BF)
    ngrp = len(groups)
    midg = (ngrp + 1) // 2
    TA = groups[midg - 1][-1] + 1 if midg < ngrp else T
    TB = T - TA
    o1t_shA = nc.dram_tensor("o1t_shA", [P, TA * P], BF)
    o1t_agA = nc.dram_tensor(
        "o1t_agA", [NC * P, TA * P], BF, addr_space="Shared"
    )
    if TB > 0:
        o1t_shB = nc.dram_tensor("o1t_shB", [P, TB * P], BF)
        o1t_agB = nc.dram_tensor(
            "o1t_agB", [NC * P, TB * P], BF, addr_space="Shared"
        )
    o2_sh = nc.dram_tensor("o2_sh", [NLOC, P], BF)
    o2_ag = nc.dram_tensor("o2_ag", [NPAD, P], BF, addr_space="Shared")

    AF = mybir.ActivationFunctionType
    OP = mybir.AluOpType

    with TileContext(nc) as tc:
        with (
            tc.tile_pool(name="const", bufs=1) as cp,
            tc.tile_pool(name="wtile", bufs=3) as wp,
            tc.tile_pool(name="htile", bufs=3) as hp,
            tc.tile_pool(name="gitile", bufs=2) as gip,
            tc.tile_pool(name="gload", bufs=4) as glp,
            tc.tile_pool(name="gevict", bufs=4) as gep,
            tc.tile_pool(name="aevict", bufs=2) as aep,
            tc.tile_pool(name="lab", bufs=2) as lp,
            tc.tile_pool(name="ps_gemm", bufs=4, space="PSUM") as psg,
            tc.tile_pool(name="ps_agg", bufs=4, space="PSUM") as psa,
        ):
            nc.gpsimd.load_library(mlp)
            # ---- persistent SBUF ----
            meta_sb = cp.tile([P, 2 * C], BF)
            nc.sync.dma_start(out=meta_sb[:], in_=meta_d[:, :])
            iota_sb = cp.tile([P, KSMAX * P], BF)
            nc.sync.dma_start(out=iota_sb[:], in_=iota_d[:, :])
            lidx_sb = cp.tile([P, lay["LICOLS"]], I16)
            nc.sync.dma_start(out=lidx_sb[:], in_=lidx_d[:, :])
            w1_sb = cp.tile([P, P], BF)
            nc.sync.dma_start(out=w1_sb[:], in_=w1_d[:, :])
            w2_sb = cp.tile([P, P], BF)
            nc.sync.dma_start(out=w2_sb[:], in_=w2_d[:, :])
            b1c_sb = cp.tile([P, 1], F32)
            nc.sync.dma_start(out=b1c_sb[:], in_=b1c_d[:, :])
            b2_sb = cp.tile([P, P], F32)
            nc.sync.dma_start(out=b2_sb[:], in_=b2_d[:, :])
            wv_sb = cp.tile([P, LABMAX * P], F32)
            nc.sync.dma_start(out=wv_sb[:], in_=wv_d[:, :])
            res_sb = cp.tile([P, LCp], F32)

            iota3 = iota_sb[:].rearrange("p (g e) -> p g e", e=P)

            def build_w(lt):
                # one-hot W for all chunks of tile lt in two batched DVE ops:
                # W[e, j, n] = (iota[n] == dstl[e,j]) * norm[e,j]
                K = Ksum[lt]
                c0 = mc0[lt]
                w = wp.tile([P, KSMAX * P], BF, tag="w")
                w3 = w[:, : K * P].rearrange("p (g e) -> p g e", e=P)
                nc.vector.tensor_tensor(
                    out=w3,
                    in0=iota3[:, :K, :],
                    in1=meta_sb[:, c0 : c0 + K].to_broadcast([P, K, P]),
                    op=OP.is_equal,
                )
                nc.vector.tensor_tensor(
                    out=w3,
                    in0=w3,
                    in1=meta_sb[:, C + c0 : C + c0 + K].to_broadcast(
                        [P, K, P]
                    ),
                    op=OP.mult,
                )
                return w

            # ---- h table GEMM pass (full table, replicated per core) ----
            def gemm_pass(layer):
                w_sb = w1_sb if layer == 1 else w2_sb
                htab = htab1 if layer == 1 else htab2
                gr = []
                if layer == 1:
                    for t0 in range(0, GT, GEMM_G):
                        gr.append((t0, min(GEMM_G, GT - t0)))
                elif layer == 2:
                    for q in range(NC):
                        for lt0 in range(0, TA, GEMM_G):
                            gr.append((q * T + lt0, min(GEMM_G, TA - lt0)))
                else:  # layer == 3: B half of layer 2
                    for q in range(NC):
                        for lt0 in range(TA, T, GEMM_G):
                            gr.append((q * T + lt0, min(GEMM_G, T - lt0)))
                for t0, gs in gr:
                    lhsT = glp.tile([P, GEMM_G * P], BF, tag="lhsT")
                    if layer == 1:
                        nc.scalar.dma_start(
                            out=lhsT[:, : gs * P],
                            in_=xT[:, t0 * P : (t0 + gs) * P],
                        )
                    else:
                        q, lt0 = divmod(t0, T)
                        agt = o1t_agA if lt0 < TA else o1t_agB
                        lb = lt0 if lt0 < TA else lt0 - TA
                        nc.scalar.dma_start(
                            out=lhsT[:, : gs * P],
                            in_=agt[
                                q * P : (q + 1) * P,
                                lb * P : (lb + gs) * P,
                            ],
                        )
                    hb = gep.tile([P, GEMM_G * P], BF, tag="hb")
                    for p0 in range(0, gs, 4):
                        pw = min(4, gs - p0)
                        pg = psg.tile([P, 512], F32)
                        for i in range(pw):
                            nc.tensor.matmul(
                                out=pg[:, i * P : (i + 1) * P],
                                lhsT=lhsT[
                                    :, (p0 + i) * P : (p0 + i + 1) * P
                                ],
                                rhs=w_sb[:],
                                start=True,
                                stop=True,
                            )
                        nc.scalar.activation(
                            hb[:, p0 * P : (p0 + pw) * P],
                            pg[:, : pw * P],
                            AF.Copy,
                        )
                    nc.sync.dma_start(
                        out=htab[t0 * P : (t0 + gs) * P, :]
                        .rearrange("(i p) j -> p i j", p=P),
                        in_=hb[:, : gs * P]
                        .rearrange("p (i j) -> p i j", j=P),
                    )

            # ---- aggregation pass over owned dst tiles ----
            def agg_pass(layer):
                htab = htab1 if layer == 1 else htab2
                for gi, tiles in enumerate(groups):
                    NCHg = nchg[gi]
                    gt = gip.tile([P, NCHMAX * 8], I16, tag="gi")
                    nc.sync.dma_start(
                        out=gt[:, : NCHg * 8],
                        in_=gidx_d[:, gcol0[gi] : gcol0[gi] + NCHg * 8],
                    )
                    h = hp.tile([P, NCHMAX * P], BF, tag="h")
                    for b in range(NBANK):
                        s0, s1 = seg[gi][b], seg[gi][b + 1]
                        for c0 in range(s0, s1, MAXCH):
                            c1 = min(c0 + MAXCH, s1)
                            nch = c1 - c0
                            nc.gpsimd.dma_gather(
                                h[:, c0 * P : c1 * P].rearrange(
                                    "p (c e) -> p c e", e=P
                                ),
                                htab[b * BR : (b + 1) * BR, :],
                                gt[:, c0 * 8 : c1 * 8],
                                nch * P,
                                nch * P,
                                P,
                                queue_num=next_q(),
                            )
                    ob = aep.tile([P, AGG_G * P], BF, tag=f"ob{layer}")
                    ags = len(tiles)
                    for s, lt in enumerate(tiles):
                        w = build_w(lt)
                        pt = psa.tile([P, P], F32)
                        K = Ksum[lt]
                        for j in range(K):
                            hs = h[
                                :,
                                hoffs[gi][lt][j] * P
                                : (hoffs[gi][lt][j] + 1) * P,
                            ]
                            ws = w[:, j * P : (j + 1) * P]
                            if layer == 1:
                                # psum = sum_j h_j^T @ W_j = [feat, node]
                                nc.tensor.matmul(
                                    out=pt[:], lhsT=hs, rhs=ws,
                                    start=(j == 0), stop=(j == K - 1),
                                )
                            else:
                                # psum = sum_j W_j^T @ h_j = [node, feat]
                                nc.tensor.matmul(
                                    out=pt[:], lhsT=ws, rhs=hs,
                                    start=(j == 0), stop=(j == K - 1),
                                )
                        if layer == 1:
                            nc.scalar.activation(
                                ob[:, s * P : (s + 1) * P],
                                pt[:],
                                AF.Relu,
                                bias=b1c_sb[:],
                            )
                        else:
                            t1 = aep.tile([P, P], F32, tag="t1")
                            nc.vector.tensor_tensor(
                                out=t1[:], in0=pt[:], in1=b2_sb[:],
                                op=OP.add,
                            )
                            nc.scalar.activation(
                                ob[:, s * P : (s + 1) * P], t1[:], AF.Relu
                            )
                    g0 = tiles[0]
                    if layer == 1:
                        if g0 < TA:
                            nc.sync.dma_start(
                                out=o1t_shA[:, g0 * P : (g0 + ags) * P],
                                in_=ob[:, : ags * P],
                            )
                        else:
                            nc.sync.dma_start(
                                out=o1t_shB[
                                    :, (g0 - TA) * P : (g0 - TA + ags) * P
                                ],
                                in_=ob[:, : ags * P],
                            )
                        if gi == min(midg + 1, ngrp - 1):
                            nc.gpsimd.collective_compute(
                                "AllGather",
                                OP.bypass,
                                replica_groups=rg,
                                ins=[o1t_shA[:, :]],
                                outs=[o1t_agA[:, :]],
                            )
                    else:
                        nc.sync.dma_start(
                            out=o2_sh[g0 * P : (g0 + ags) * P, :]
                            .rearrange("(i p) j -> p i j", p=P),
                            in_=ob[:, : ags * P]
                            .rearrange("p (i j) -> p i j", j=P),
                        )

            if phase >= 2:
                gemm_pass(1)
            if phase == 2:
                hprobe = cp.tile([P, P], BF)
                nc.sync.dma_start(out=hprobe[:], in_=htab1[0:P, :])
                probe_f = cp.tile([P, P], F32)
                nc.vector.tensor_copy(probe_f[:], hprobe[:])
                pb = min(LCp, P)
                nc.sync.dma_start(out=res_d[:, :pb], in_=probe_f[:, :pb])
            if phase >= 3:
                agg_pass(1)
            if phase == 3:
                oprobe = cp.tile([P, P], BF)
                nc.sync.dma_start(out=oprobe[:], in_=o1t_shA[:, 0:P])
                oprobe_f = cp.tile([P, P], F32)
                nc.vector.tensor_copy(oprobe_f[:], oprobe[:])
                pb = min(LCp, P)
                nc.sync.dma_start(out=res_d[:, :pb], in_=oprobe_f[:, :pb])
            if phase >= 4 and TB > 0:
                nc.gpsimd.collective_compute(
                    "AllGather",
                    OP.bypass,
                    replica_groups=rg,
                    ins=[o1t_shB[:, :]],
                    outs=[o1t_agB[:, :]],
                )
            if phase == 4:
                oprobe = cp.tile([P, P], BF)
                nc.sync.dma_start(out=oprobe[:], in_=o1t_agA[0:P, 0:P])
                oprobe_f = cp.tile([P, P], F32)
                nc.vector.tensor_copy(oprobe_f[:], oprobe[:])
                pb = min(LCp, P)
                nc.sync.dma_start(out=res_d[:, :pb], in_=oprobe_f[:, :pb])
            if phase >= 5:
                gemm_pass(2)
                if TB > 0:
                    gemm_pass(3)
                agg_pass(2)
                nc.gpsimd.collective_compute(
                    "AllGather",
                    OP.bypass,
                    replica_groups=rg,
                    ins=[o2_sh[:, :]],
                    outs=[o2_ag[:, :]],
                )

            # ---- label pass ----
            if phase >= 6:
                for bp in range(NBANK * NBANK):
                    nch = lkb[bp]
                    if nch == 0:
                        continue
                    b0, b1 = divmod(bp, NBANK)
                    a = lp.tile([P, LABMAX * P], BF, tag="a")
                    b = lp.tile([P, LABMAX * P], BF, tag="b")
                    for tile_, bank, col0 in (
                        (a, b0, lcol0a[bp]),
                        (b, b1, lcol0b[bp]),
                    ):
                        for c0 in range(0, nch, MAXCH):
                            c1 = min(c0 + MAXCH, nch)
                            nc.gpsimd.dma_gather(
                                tile_[:, c0 * P : c1 * P].rearrange(
                                    "p (c e) -> p c e", e=P
                                ),
                                o2_ag[bank * BR : (bank + 1) * BR, :],
                                lidx_sb[:, col0 + c0 * 8 : col0 + c1 * 8],
                                (c1 - c0) * P,
                                (c1 - c0) * P,
                                P,
                                queue_num=next_q(),
                            )
                    prod = lp.tile([P, LABMAX * P], F32, tag="prod")
                    nc.vector.tensor_tensor(
                        out=prod[:, : nch * P],
                        in0=a[:, : nch * P],
                        in1=b[:, : nch * P],
                        op=OP.mult,
                    )
                    nc.vector.tensor_tensor(
                        out=prod[:, : nch * P],
                        in0=prod[:, : nch * P],
                        in1=wv_sb[:, : nch * P],
                        op=OP.mult,
                    )
                    nc.vector.reduce_sum(
                        res_sb[:, lchunk0[bp] : lchunk0[bp] + nch],
                        prod[:, : nch * P].rearrange(
                            "p (g e) -> p g e", e=P
                        ),
                        axis=mybir.AxisListType.X,
                    )
                nc.vector.tensor_scalar_add(
                    res_sb[:], res_sb[:], float(linb_sum)
                )
                nc.sync.dma_start(out=res_d[:, :], in_=res_sb[:])

    nc.finalize()
    return nc


# ------------------------------------------------------------------ driver


def make_in_maps(cfg, prep, W1, b1, W2, b2, lin_W, lin_b):
    wv = lin_W.astype(np.float32).sum(axis=1)
    lay = prep["layout"]
    consts = dict(
        xT=prep["xT"],
        iota=prep["iota_rep"],
        w1=W1.astype(np.float32).astype(ml_dtypes.bfloat16),
        w2=W2.astype(np.float32).astype(ml_dtypes.bfloat16),
        b1c=b1.astype(np.float32).reshape(P, 1),
        b2bc=np.tile(b2.astype(np.float32)[None, :], (P, 1)),
        wvrep=np.tile(wv[None, :], (P, lay["LABMAX"])),
    )
    in_maps = []
    for q in range(NC):
        m = dict(consts)
        m.update(
            gidx=prep["gidx"][q],
            meta=prep["meta"][q],
            lidx=prep["lidx"][q],
        )
        in_maps.append(m)
    return in_maps


def assemble_output(cfg, prep, results):
    out = np.zeros(cfg.n_labels, np.float32)
    order_arr = prep["order_arr"]
    for q in range(NC):
        r = np.asarray(results[q]["res"], np.float32)  # [128, LCp]
        v = r.T.reshape(-1)  # slot-major
        m = order_arr[q] >= 0
        out[order_arr[q][m]] = v[m]
    return out


def run(cfg, x, edge_index, edge_weight, edge_label_index,
        W1, b1, W2, b2, lin_W, lin_b, trace=False, phase=99):
    global LAST_EXEC_NS, LAST_RESULTS
    prep = preprocess(cfg, np.asarray(x), np.asarray(edge_index),
                      np.asarray(edge_weight), np.asarray(edge_label_index))
    linb_sum = float(np.asarray(lin_b, np.float64).sum())
    nc = build_program(cfg, prep["layout"], linb_sum, phase=phase)
    in_maps = make_in_maps(cfg, prep, W1, b1, W2, b2, lin_W, lin_b)
    res = run_bass_kernel_spmd(
        nc, in_maps, list(range(NC)), trace=trace
    )
    LAST_EXEC_NS = res.exec_time_ns
    LAST_RESULTS = res
    return assemble_output(cfg, prep, res.results)


def kernel(x, edge_index, edge_weight, edge_label_index,
           W1, b1, W2, b2, lin_W, lin_b):
    trace = bool(os.environ.get("KERNEL_TRACE"))
    return run(FULL, x, edge_index, edge_weight, edge_label_index,
               W1, b1, W2, b2, lin_W, lin_b, trace=trace)



# revision 5
# speedup vs baseline: 1.1999x; 1.1999x over previous
"""GCN link-predictor kernel for 8 Trainium2 NeuronCores (Bass/Tile).

v2 — gather-wall-optimized SPMD design (single program, 8 cores):

  Per-core SWDGE dma_gather sustains ~111 GB/s (4 queues x ~30 GB/s,
  descriptor-dispatch-bound at 256B rows), so the kernel is organized to
  (a) minimize gathered rows and (b) keep everything else off the
  critical path:

  - Nodes: core q owns orig rows [q*12500,(q+1)*12500), padded to 12800
    (100 tiles of 128).  The global padded table is QUARTER-INTERLEAVED:
    node (q, quarter k, row r) lives at bank k (25600 rows < 2^15, int16
    gather indices), position q*3200 + r.  Each bank is produced by its
    own AllGather so gathers pipeline behind the collectives.
  - GEMM1 is sharded: each core computes h1 = x_shard @ W1 only for its
    own 12800 rows (vs full-table replicated in v1), stages to DRAM and
    AllGathers per quarter.
  - Aggregation (per layer): edges grouped per (dst-group of 5 tiles,
    src bank).  Within a (g,b) stream, slots are laid out per tile with
    uniform cross-core offsets (len = max over cores), padded only at
    stream ends -> ~7% padding (vs ~28% for per-(tile,bank) chunks).
    One-hot W columns are built per (chunk, tile) with norm-masked
    entries (pad/foreign slots have norm 0), batched in 2 DVE ops per
    (g,b).  PSUM accumulates K matmuls per dst tile; self-loops use
    SBUF-resident own-shard h tiles (zero gather descriptors).
  - Layer-2 GEMM is fused: the relu'd layer-1 psum tile (feature-major)
    is fed straight into one matmul with W2 producing the node-major h2
    tile; h2 is staged + AllGathered per quarter while agg1 continues.
  - Labels: 16 streams by (bank(a), bank(b)); both rows gathered
    node-major, product + lin_W-row-sum reduce on DVE.
"""

import os
import sys

import numpy as np

for _p in ("/opt/trn_rl_repo",):
    if _p not in sys.path:
        sys.path.insert(0, _p)

import ml_dtypes  # noqa: E402

import concourse.bacc as bacc  # noqa: E402
import concourse.bass as bass  # noqa: E402
import concourse.mybir as mybir  # noqa: E402
from concourse.bass_utils import run_bass_kernel_spmd  # noqa: E402
from concourse.library_config import mlp  # noqa: E402
from concourse.tile import TileContext  # noqa: E402

P = 128
NC = 8
NBANK = 4
BF = mybir.dt.bfloat16
F32 = mybir.dt.float32
I16 = mybir.dt.int16

LAST_EXEC_NS = None
LAST_RESULTS = None

MAXCH = 8
GMAX = 4  # max dst tiles per aggregation group (one PSUM bank)


class Cfg:
    def __init__(self, n_nodes, n_labels):
        assert n_nodes % NC == 0
        self.n_nodes = n_nodes
        self.nodes_per_core = n_nodes // NC
        # pad shard to a multiple of 4 quarters of whole tiles
        self.tiles_per_core = -(-self.nodes_per_core // (4 * P)) * 4
        self.n_loc = self.tiles_per_core * P
        self.n_pad = NC * self.n_loc
        self.qrows = self.n_loc // 4          # shard rows per quarter
        self.bank_rows = NC * self.qrows      # rows per bank (= quarter)
        assert self.bank_rows < (1 << 15)
        self.qtiles = self.tiles_per_core // 4
        self.n_labels = n_labels
        self.lab_per_core = -(-n_labels // NC)


FULL = Cfg(100000, 200000)


def _groups_of(cfg):
    """Group sizes per quarter (each <= GMAX), tiled over 4 quarters."""
    qt = cfg.qtiles
    sizes = []
    r = qt
    while r > 0:
        s = min(GMAX, r)
        sizes.append(s)
        r -= s
    groups = []  # list of (tile0, ntiles) for the WHOLE shard
    for k in range(4):
        t0 = k * qt
        for s in sizes:
            groups.append((t0, s))
            t0 += s
    return groups


# ---------------------------------------------------------------- host prep


def _place(cfg, ids):
    """orig node id -> (core, shard_row, bank, bank_row)."""
    q = np.minimum(ids // cfg.nodes_per_core, NC - 1)
    r = ids - q * cfg.nodes_per_core
    k = r // cfg.qrows
    br = q * cfg.qrows + (r - k * cfg.qrows)
    return q, r, k, br


def preprocess(cfg, x, edge_index, edge_weight, edge_label_index):
    n = cfg.n_nodes
    T = cfg.tiles_per_core
    groups = _groups_of(cfg)
    NG = len(groups)

    src = edge_index[0].astype(np.int64)
    dst = edge_index[1].astype(np.int64)
    ew = edge_weight.astype(np.float32)
    # symmetric GCN normalization incl self loops (host scalar prep)
    deg = np.bincount(dst, weights=ew, minlength=n).astype(np.float32)
    deg += 1.0  # self loop weight
    dinv = (1.0 / np.sqrt(np.maximum(deg, 1e-12))).astype(np.float32)
    norm = dinv[src] * ew * dinv[dst]

    sq, sr, sk, sbr = _place(cfg, src)
    dq, dr, _, _ = _place(cfg, dst)
    d_tile = dr // P
    d_loc = dr % P

    # group id of each dst tile
    tile2g = np.zeros(T, np.int64)
    for gi, (t0, s) in enumerate(groups):
        tile2g[t0: t0 + s] = gi
    d_g = tile2g[d_tile]

    # ---- per (core, group, bank, tile) counts -> uniform slot layout ----
    NKEY = NG * NBANK * T
    key_t = (d_g * NBANK + sk) * T + d_tile
    key_full = (dq * NG * NBANK + d_g * NBANK + sk) * T + d_tile
    cnt = np.bincount(key_full, minlength=NC * NKEY).reshape(NC, NG, NBANK, T)
    tlen = cnt.max(axis=0)  # [NG, NBANK, T] uniform per-tile slot lengths

    # stream/segment layout (identical across cores)
    #  per (g,b): tiles t0..t0+s-1 at offsets off[t], total padded to 128
    nch = np.zeros((NG, NBANK), np.int64)      # chunks per (g,b)
    toff = np.zeros((NG, NBANK, T), np.int64)  # slot offset of tile in stream
    c0 = np.zeros((NG, NBANK), np.int64)       # first chunk col of (g,b)
    segs = [[None] * NBANK for _ in range(NG)]  # per (g,b): list of seg dicts
    ct0 = 0
    CT_cols = []   # flat mdst/mnrm segment column count
    cseq = 0
    for gi, (t0, s) in enumerate(groups):
        for b in range(NBANK):
            off = 0
            sl = []
            for t in range(t0, t0 + s):
                toff[gi, b, t] = off
                off += int(tlen[gi, b, t])
            tot = off
            ch = -(-tot // P) if tot else 0
            nch[gi, b] = ch
            c0[gi, b] = cseq
            cseq += ch
            # segments: (tile, chunk, ct_col)
            for t in range(t0, t0 + s):
                lo, hi = int(toff[gi, b, t]), int(toff[gi, b, t] + tlen[gi, b, t])
                if hi == lo:
                    continue
                for c in range(lo // P, -(-hi // P)):
                    sl.append((t, c, ct0))
                    ct0 += 1
            segs[gi][b] = sl
    CE = cseq          # total edge chunks per core
    CT = ct0           # total masked meta columns
    NCTMAX = max(
        (len(segs[gi][b]) for gi in range(NG) for b in range(NBANK)),
        default=1,
    )
    NCHMAX = int(nch.max())

    # ---- place edges into slots ----
    order = np.lexsort((src, key_t, dq))  # by (core, g, b, tile), stable
    so = order
    core_o = dq[so]
    g_o = d_g[so]
    b_o = sk[so]
    t_o = d_tile[so]
    # position within (core,g,b,t)
    sk2 = (core_o * NG * NBANK + g_o * NBANK + b_o) * T + t_o
    starts = np.zeros(NC * NKEY + 1, np.int64)
    starts[1:] = np.cumsum(cnt.reshape(-1))
    pos_in = np.arange(len(so)) - starts[sk2]
    slot = toff[g_o, b_o, t_o] + pos_in            # slot within (g,b) stream
    chunk = c0[g_o, b_o] + slot // P               # global chunk col
    part = slot % P

    gidx16 = np.zeros((NC, 16, CE * 8), np.int16)
    gidx16[core_o, part % 16, chunk * 8 + part // 16] = sbr[so].astype(np.int16)
    gidx = np.ascontiguousarray(np.tile(gidx16, (1, 8, 1)))

    # masked per-(chunk,tile) meta columns
    mdst_a = np.zeros((NC, P, CT), np.float32)
    mnrm_a = np.zeros((NC, P, CT), np.float32)
    # vectorized edge -> ct col: ct = ct_base[g,b,t] + slot//P - clo[g,b,t]
    ct_base = np.zeros((NG, NBANK, T), np.int64)
    clo_arr = np.zeros((NG, NBANK, T), np.int64)
    for gi in range(NG):
        for b in range(NBANK):
            for (t, c, ct) in segs[gi][b]:
                if ct_base[gi, b, t] == 0 and clo_arr[gi, b, t] == 0:
                    ct_base[gi, b, t] = ct
                    clo_arr[gi, b, t] = c
                else:
                    ct_base[gi, b, t] = min(ct_base[gi, b, t], ct)
                    clo_arr[gi, b, t] = min(clo_arr[gi, b, t], c)
    ct_of = (ct_base[g_o, b_o, t_o] + slot // P - clo_arr[g_o, b_o, t_o])
    mdst_a[core_o, part, ct_of] = d_loc[so]
    mnrm_a[core_o, part, ct_of] = norm[so]
    mdst = mdst_a.astype(ml_dtypes.bfloat16)
    mnrm = mnrm_a.astype(ml_dtypes.bfloat16)

    # per-tile self-loop scale dinv^2 (0 on padded rows)
    mself_a = np.zeros((NC, P, T), np.float32)
    for q in range(NC):
        lo = q * cfg.nodes_per_core
        cnt_q = cfg.nodes_per_core
        d2 = dinv[lo: lo + cnt_q] ** 2
        full = np.zeros(cfg.n_loc, np.float32)
        full[:cnt_q] = d2
        mself_a[q] = full.reshape(T, P).T
    mself = mself_a.astype(ml_dtypes.bfloat16)

    # ---- labels: 16 streams by (bank(a), bank(b)) ----
    ela = edge_label_index[0].astype(np.int64)
    elb = edge_label_index[1].astype(np.int64)
    _, _, ak, abr = _place(cfg, ela)
    _, _, bk, bbr = _place(cfg, elb)
    bp_all = ak * NBANK + bk
    lpc = cfg.lab_per_core
    NBP = NBANK * NBANK
    cnts = np.zeros((NC, NBP), np.int64)
    for q in range(NC):
        lo, hi = q * lpc, min((q + 1) * lpc, cfg.n_labels)
        cnts[q] = np.bincount(bp_all[lo:hi], minlength=NBP)
    lkb = (-(-cnts.max(axis=0) // P)).astype(np.int64)
    lchunk0 = np.zeros(NBP + 1, np.int64)
    lchunk0[1:] = np.cumsum(lkb)
    LCp = int(lchunk0[-1])
    LABMAX = int(lkb.max())
    lcol0a = lchunk0[:-1] * 16
    lcol0b = lcol0a + lkb * 8
    LICOLS = LCp * 16

    lidx16 = np.zeros((NC, 16, LICOLS), np.int16)
    order_arr = np.full((NC, LCp * P), -1, np.int64)
    for q in range(NC):
        lo, hi = q * lpc, min((q + 1) * lpc, cfg.n_labels)
        bp_q = bp_all[lo:hi]
        oq = np.argsort(bp_q, kind="stable")
        sbp = bp_q[oq]
        st = np.zeros(NBP + 1, np.int64)
        st[1:] = np.cumsum(cnts[q])
        pos = np.arange(len(oq)) - st[sbp]
        cola = lcol0a[sbp] + pos // 16
        colb = lcol0b[sbp] + pos // 16
        prt = pos % 16
        lidx16[q, prt, cola] = abr[lo:hi][oq].astype(np.int16)
        lidx16[q, prt, colb] = bbr[lo:hi][oq].astype(np.int16)
        sl = (lchunk0[sbp] + pos // P) * P + pos % P
        order_arr[q, sl] = lo + oq
    lidx = np.ascontiguousarray(np.tile(lidx16, (1, 8, 1)))

    # ---- node features shards, feature-major ----
    xT_shards = []
    for q in range(NC):
        lo = q * cfg.nodes_per_core
        xs = np.zeros((cfg.n_loc, P), np.float32)
        xs[: cfg.nodes_per_core] = x[lo: lo + cfg.nodes_per_core]
        xT_shards.append(
            np.ascontiguousarray(xs.T).astype(ml_dtypes.bfloat16)
        )

    iota_rep = np.tile(
        np.arange(P, dtype=np.float32)[None, :], (P, max(NCTMAX, 1))
    ).astype(ml_dtypes.bfloat16)
    iota_col = np.arange(P, dtype=np.float32).reshape(P, 1).astype(
        ml_dtypes.bfloat16
    )

    layout = dict(
        groups=groups, nch=nch, c0=c0, segs=segs, CE=CE, CT=CT,
        NCTMAX=NCTMAX, NCHMAX=NCHMAX,
        lkb=[int(v) for v in lkb], lchunk0=[int(v) for v in lchunk0],
        LCp=LCp, LABMAX=LABMAX,
        lcol0a=[int(v) for v in lcol0a], lcol0b=[int(v) for v in lcol0b],
        LICOLS=LICOLS,
    )
    return dict(gidx=gidx, mdst=mdst, mnrm=mnrm, mself=mself, lidx=lidx,
                xT_shards=xT_shards, iota_rep=iota_rep, iota_col=iota_col,
                order_arr=order_arr, layout=layout)


# ------------------------------------------------------------- bass program


def build_program(cfg, lay, linb_sum, phase=99):
    T = cfg.tiles_per_core
    QR = cfg.qrows
    BR = cfg.bank_rows
    QT = cfg.qtiles
    groups = lay["groups"]
    NG = len(groups)
    nch, c0g, segs = lay["nch"], lay["c0"], lay["segs"]
    CE, CT, NCTMAX, NCHMAX = lay["CE"], lay["CT"], lay["NCTMAX"], lay["NCHMAX"]
    LCp, LABMAX = lay["LCp"], lay["LABMAX"]
    lkb, lchunk0 = lay["lkb"], lay["lchunk0"]
    lcol0a, lcol0b = lay["lcol0a"], lay["lcol0b"]
    rg = [list(range(NC))]

    nc = bacc.Bacc(None, target_bir_lowering=False, debug=False,
                   dynamic_dma_scratch_size=16384, num_swdge_queues=4)
    qrr = [0]

    def next_q():
        qrr[0] = (qrr[0] + 1) % 4
        return qrr[0]

    xT_d = nc.declare_dram_parameter("xTq", [P, cfg.n_loc], BF, False)
    gidx_d = nc.declare_dram_parameter("gidx", [P, CE * 8], I16, False)
    mdst_d = nc.declare_dram_parameter("mdst", [P, CT], BF, False)
    mnrm_d = nc.declare_dram_parameter("mnrm", [P, CT], BF, False)
    mself_d = nc.declare_dram_parameter("mself", [P, T], BF, False)
    iota_d = nc.declare_dram_parameter("iota", [P, NCTMAX * P], BF, False)
    iotac_d = nc.declare_dram_parameter("iotac", [P, 1], BF, False)
    lidx_d = nc.declare_dram_parameter("lidx", [P, lay["LICOLS"]], I16, False)
    w1_d = nc.declare_dram_parameter("w1", [P, P], BF, False)
    w2_d = nc.declare_dram_parameter("w2", [P, P], BF, False)
    b1c_d = nc.declare_dram_parameter("b1c", [P, 1], F32, False)
    b2_d = nc.declare_dram_parameter("b2bc", [P, P], F32, False)
    wv_d = nc.declare_dram_parameter("wvrep", [P, LABMAX * P], F32, False)
    res_d = nc.declare_dram_parameter("res", [P, LCp], F32, True)

    h1_sh = [nc.dram_tensor(f"h1sh{k}", [QR, P], BF) for k in range(4)]
    h2_sh = [nc.dram_tensor(f"h2sh{k}", [QR, P], BF) for k in range(4)]
    o2_sh = [nc.dram_tensor(f"o2sh{k}", [QR, P], BF) for k in range(4)]
    htab1 = [
        nc.dram_tensor(f"htab1_{k}", [BR, P], BF, addr_space="Shared")
        for k in range(4)
    ]
    htab2 = [
        nc.dram_tensor(f"htab2_{k}", [BR, P], BF, addr_space="Shared")
        for k in range(4)
    ]
    o2tab = [
        nc.dram_tensor(f"o2tab_{k}", [BR, P], BF, addr_space="Shared")
        for k in range(4)
    ]

    AF = mybir.ActivationFunctionType
    OP = mybir.AluOpType

    with TileContext(nc) as tc:
        with (
            tc.tile_pool(name="const", bufs=1) as cp,
            tc.tile_pool(name="xload", bufs=2) as xp,
            tc.tile_pool(name="gemmev", bufs=3) as gep,
            tc.tile_pool(name="gitile", bufs=2) as gip,
            tc.tile_pool(name="hgb", bufs=6) as hp,
            tc.tile_pool(name="wgb", bufs=4) as wp,
            tc.tile_pool(name="aggev", bufs=3) as aep,
            tc.tile_pool(name="lab", bufs=2) as lp,
            tc.tile_pool(name="ps_gemm", bufs=2, space="PSUM") as psg,
            tc.tile_pool(name="ps_agg", bufs=3, space="PSUM") as psa,
        ):
            nc.gpsimd.load_library(mlp)
            # ---- persistent SBUF ----
            mdst_sb = cp.tile([P, CT], BF)
            nc.sync.dma_start(out=mdst_sb[:], in_=mdst_d[:, :])
            mnrm_sb = cp.tile([P, CT], BF)
            nc.sync.dma_start(out=mnrm_sb[:], in_=mnrm_d[:, :])
            mself_sb = cp.tile([P, T], BF)
            nc.sync.dma_start(out=mself_sb[:], in_=mself_d[:, :])
            iota_sb = cp.tile([P, NCTMAX * P], BF)
            nc.sync.dma_start(out=iota_sb[:], in_=iota_d[:, :])
            iotac_sb = cp.tile([P, 1], BF)
            nc.sync.dma_start(out=iotac_sb[:], in_=iotac_d[:, :])
            lidx_sb = cp.tile([P, lay["LICOLS"]], I16)
            nc.sync.dma_start(out=lidx_sb[:], in_=lidx_d[:, :])
            w1_sb = cp.tile([P, P], BF)
            nc.sync.dma_start(out=w1_sb[:], in_=w1_d[:, :])
            w2_sb = cp.tile([P, P], BF)
            nc.sync.dma_start(out=w2_sb[:], in_=w2_d[:, :])
            b1c_sb = cp.tile([P, 1], F32)
            nc.sync.dma_start(out=b1c_sb[:], in_=b1c_d[:, :])
            b2_sb = cp.tile([P, P], F32)
            nc.sync.dma_start(out=b2_sb[:], in_=b2_d[:, :])
            wv_sb = cp.tile([P, LABMAX * P], F32)
            nc.sync.dma_start(out=wv_sb[:], in_=wv_d[:, :])
            res_sb = cp.tile([P, LCp], F32)

            selfh1 = cp.tile([P, T * P], BF)   # dinv^2-scaled own h1 tiles
            selfh2 = cp.tile([P, T * P], BF)
            # identity one-hot for self chunks
            idmat = cp.tile([P, P], BF)
            nc.vector.tensor_tensor(
                out=idmat[:],
                in0=iota_sb[:, :P],
                in1=iotac_sb[:].to_broadcast([P, P]),
                op=OP.is_equal,
            )

            iota3 = iota_sb[:].rearrange("p (g e) -> p g e", e=P)

            # ---- sharded GEMM1 + quarter AllGathers ----
            def gemm1():
                G1 = 4
                for k in range(4):
                    for t0 in range(k * QT, (k + 1) * QT, G1):
                        gs = min(G1, (k + 1) * QT - t0)
                        lhsT = xp.tile([P, 4 * P], BF, tag="x")
                        nc.scalar.dma_start(
                            out=lhsT[:, : gs * P],
                            in_=xT_d[:, t0 * P: (t0 + gs) * P],
                        )
                        pg = psg.tile([P, 4 * P], F32, tag="g1")
                        for i in range(gs):
                            nc.tensor.matmul(
                                out=pg[:, i * P: (i + 1) * P],
                                lhsT=lhsT[:, i * P: (i + 1) * P],
                                rhs=w1_sb[:],
                                start=True, stop=True,
                            )
                        hb = gep.tile([P, 4 * P], BF, tag="hb")
                        nc.scalar.activation(
                            hb[:, : gs * P], pg[:, : gs * P], AF.Copy
                        )
                        # self-scaled copy (dinv^2 per node row)
                        nc.vector.tensor_tensor(
                            out=selfh1[:, t0 * P: (t0 + gs) * P]
                            .rearrange("p (g e) -> p g e", e=P),
                            in0=hb[:, : gs * P]
                            .rearrange("p (g e) -> p g e", e=P),
                            in1=mself_sb[:, t0: t0 + gs]
                            .to_broadcast([P, gs, P]),
                            op=OP.mult,
                        )
                        nc.sync.dma_start(
                            out=h1_sh[k][
                                (t0 - k * QT) * P: (t0 - k * QT + gs) * P, :
                            ].rearrange("(i p) j -> p i j", p=P),
                            in_=hb[:, : gs * P]
                            .rearrange("p (i j) -> p i j", j=P),
                        )
                    nc.gpsimd.collective_compute(
                        "AllGather", OP.bypass, replica_groups=rg,
                        ins=[h1_sh[k][:, :]], outs=[htab1[k][:, :]],
                    )

            # ---- aggregation layer (fused gemm2 after layer 1) ----
            def agg(layer):
                htab = htab1 if layer == 1 else htab2
                sh = selfh1 if layer == 1 else selfh2
                for gi, (t0, gs) in enumerate(groups):
                    chg = int(nch[gi].sum())
                    if chg:
                        gt = gip.tile([P, NCHMAX * NBANK * 8], I16, tag="gi")
                        nc.sync.dma_start(
                            out=gt[:, : chg * 8],
                            in_=gidx_d[
                                :, c0g[gi, 0] * 8: (c0g[gi, 0] + chg) * 8
                            ],
                        )
                    # gathers per bank
                    htiles = {}
                    for b in range(NBANK):
                        ch = int(nch[gi, b])
                        if ch == 0:
                            continue
                        h = hp.tile([P, NCHMAX * P], BF, tag="h")
                        htiles[b] = h
                        gtoff = (c0g[gi, b] - c0g[gi, 0]) * 8
                        for cc in range(0, ch, MAXCH):
                            cw = min(MAXCH, ch - cc)
                            nc.gpsimd.dma_gather(
                                h[:, cc * P: (cc + cw) * P].rearrange(
                                    "p (c e) -> p c e", e=P
                                ),
                                htab[b][:, :],
                                gt[:, gtoff + cc * 8: gtoff + (cc + cw) * 8],
                                cw * P, cw * P, P,
                                queue_num=next_q(),
                            )
                    # build all W tiles for the group first (DVE)
                    wtiles = {}
                    ct_firsts = {}
                    for b in range(NBANK):
                        sl = segs[gi][b]
                        if not sl:
                            continue
                        nct = len(sl)
                        ct_first = sl[0][2]
                        ct_firsts[b] = ct_first
                        w = wp.tile([P, NCTMAX * P], BF, tag="w")
                        wtiles[b] = w
                        w3 = w[:, : nct * P].rearrange(
                            "p (g e) -> p g e", e=P
                        )
                        nc.vector.tensor_tensor(
                            out=w3,
                            in0=iota3[:, :nct, :],
                            in1=mdst_sb[:, ct_first: ct_first + nct]
                            .to_broadcast([P, nct, P]),
                            op=OP.is_equal,
                        )
                        nc.vector.tensor_tensor(
                            out=w3,
                            in0=w3,
                            in1=mnrm_sb[:, ct_first: ct_first + nct]
                            .to_broadcast([P, nct, P]),
                            op=OP.mult,
                        )
                    # per-tile segment lists: a psum region's accumulation
                    # must run start->stop contiguously (interleaved open
                    # accumulations within one bank corrupt results)
                    tsegs = {t: [] for t in range(t0, t0 + gs)}
                    for b in range(NBANK):
                        if segs[gi][b]:
                            for (t, c, ct) in segs[gi][b]:
                                tsegs[t].append((b, c, ct))
                    pgrp = psa.tile([P, GMAX * P], F32)
                    pts = {}
                    for i, t in enumerate(range(t0, t0 + gs)):
                        pts[t] = pgrp[:, i * P: (i + 1) * P]
                        shs = sh[:, t * P: (t + 1) * P]
                        onlyself = not tsegs[t]
                        if layer == 1:
                            nc.tensor.matmul(
                                out=pts[t], lhsT=shs, rhs=idmat[:],
                                start=True, stop=onlyself,
                            )
                        else:
                            nc.tensor.matmul(
                                out=pts[t], lhsT=idmat[:], rhs=shs,
                                start=True, stop=onlyself,
                            )
                        nseg = len(tsegs[t])
                        for si, (b, c, ct) in enumerate(tsegs[t]):
                            hs = htiles[b][:, c * P: (c + 1) * P]
                            ws = wtiles[b][
                                :, (ct - ct_firsts[b]) * P:
                                (ct - ct_firsts[b] + 1) * P]
                            stop = si == nseg - 1
                            if layer == 1:
                                nc.tensor.matmul(
                                    out=pts[t], lhsT=hs, rhs=ws,
                                    start=False, stop=stop,
                                )
                            else:
                                nc.tensor.matmul(
                                    out=pts[t], lhsT=ws, rhs=hs,
                                    start=False, stop=stop,
                                )
                    # evictions
                    k = t0 // QT
                    tq0 = t0 - k * QT
                    if layer == 1:
                        # relu(psum + b1) feature-major -> fused gemm2 ->
                        # node-major h2 tile
                        ob = aep.tile([P, GMAX * P], BF, tag="ob")
                        h2b = aep.tile([P, GMAX * P], BF, tag="h2b")
                        for i, t in enumerate(range(t0, t0 + gs)):
                            nc.scalar.activation(
                                ob[:, i * P: (i + 1) * P], pts[t],
                                AF.Relu, bias=b1c_sb[:],
                            )
                        for i, t in enumerate(range(t0, t0 + gs)):
                            pg2 = psg.tile([P, P], F32, tag="g2")
                            nc.tensor.matmul(
                                out=pg2[:],
                                lhsT=ob[:, i * P: (i + 1) * P],
                                rhs=w2_sb[:],
                                start=True, stop=True,
                            )
                            nc.scalar.activation(
                                h2b[:, i * P: (i + 1) * P], pg2[:], AF.Copy
                            )
                        nc.vector.tensor_tensor(
                            out=selfh2[:, t0 * P: (t0 + gs) * P]
                            .rearrange("p (g e) -> p g e", e=P),
                            in0=h2b[:, : gs * P]
                            .rearrange("p (g e) -> p g e", e=P),
                            in1=mself_sb[:, t0: t0 + gs]
                            .to_broadcast([P, gs, P]),
                            op=OP.mult,
                        )
                        nc.sync.dma_start(
                            out=h2_sh[k][tq0 * P: (tq0 + gs) * P, :]
                            .rearrange("(i p) j -> p i j", p=P),
                            in_=h2b[:, : gs * P]
                            .rearrange("p (i j) -> p i j", j=P),
                        )
                    else:
                        # node-major psum: +b2 (DVE), relu (scalar)
                        o2b = aep.tile([P, GMAX * P], BF, tag="o2b")
                        for i, t in enumerate(range(t0, t0 + gs)):
                            t1 = aep.tile([P, P], F32, tag="t1")
                            nc.vector.tensor_tensor(
                                out=t1[:], in0=pts[t], in1=b2_sb[:],
                                op=OP.add,
                            )
                            nc.scalar.activation(
                                o2b[:, i * P: (i + 1) * P], t1[:], AF.Relu
                            )
                        nc.sync.dma_start(
                            out=o2_sh[k][tq0 * P: (tq0 + gs) * P, :]
                            .rearrange("(i p) j -> p i j", p=P),
                            in_=o2b[:, : gs * P]
                            .rearrange("p (i j) -> p i j", j=P),
                        )
                    if tq0 + gs == QT:  # quarter complete
                        if layer == 1:
                            nc.gpsimd.collective_compute(
                                "AllGather", OP.bypass, replica_groups=rg,
                                ins=[h2_sh[k][:, :]], outs=[htab2[k][:, :]],
                            )
                        else:
                            nc.gpsimd.collective_compute(
                                "AllGather", OP.bypass, replica_groups=rg,
                                ins=[o2_sh[k][:, :]], outs=[o2tab[k][:, :]],
                            )

            def labels():
                NBP = NBANK * NBANK
                bporder = sorted(range(NBP), key=lambda bp: max(
                    bp // NBANK, bp % NBANK))
                for bp in bporder:
                    nchb = lkb[bp]
                    if nchb == 0:
                        continue
                    b0, b1 = divmod(bp, NBANK)
                    a = lp.tile([P, LABMAX * P], BF, tag="a")
                    bb = lp.tile([P, LABMAX * P], BF, tag="b")
                    for tile_, bank, col0 in (
                        (a, b0, lcol0a[bp]),
                        (bb, b1, lcol0b[bp]),
                    ):
                        for cc in range(0, nchb, MAXCH):
                            cw = min(MAXCH, nchb - cc)
                            nc.gpsimd.dma_gather(
                                tile_[:, cc * P: (cc + cw) * P].rearrange(
                                    "p (c e) -> p c e", e=P
                                ),
                                o2tab[bank][:, :],
                                lidx_sb[:, col0 + cc * 8: col0 + (cc + cw) * 8],
                                cw * P, cw * P, P,
                                queue_num=next_q(),
                            )
                    prod = lp.tile([P, LABMAX * P], F32, tag="prod")
                    nc.vector.tensor_tensor(
                        out=prod[:, : nchb * P],
                        in0=a[:, : nchb * P],
                        in1=bb[:, : nchb * P],
                        op=OP.mult,
                    )
                    nc.vector.tensor_tensor(
                        out=prod[:, : nchb * P],
                        in0=prod[:, : nchb * P],
                        in1=wv_sb[:, : nchb * P],
                        op=OP.mult,
                    )
                    nc.vector.reduce_sum(
                        res_sb[:, lchunk0[bp]: lchunk0[bp] + nchb],
                        prod[:, : nchb * P].rearrange(
                            "p (g e) -> p g e", e=P
                        ),
                        axis=mybir.AxisListType.X,
                    )
                nc.vector.tensor_scalar_add(
                    res_sb[:], res_sb[:], float(linb_sum)
                )
                nc.sync.dma_start(out=res_d[:, :], in_=res_sb[:])

            def probe(src_ap, cast=True):
                prb = cp.tile([P, P], F32)
                if cast:
                    tmp = cp.tile([P, P], BF)
                    nc.sync.dma_start(out=tmp[:], in_=src_ap)
                    nc.vector.tensor_copy(prb[:], tmp[:])
                pb = min(LCp, P)
                nc.sync.dma_start(out=res_d[:, :pb], in_=prb[:, :pb])

            if phase >= 2:
                gemm1()
            if phase == 2:
                probe(htab1[0][0:P, :])
            if phase >= 3:
                agg(1)
            if phase == 3:
                probe(htab2[0][0:P, :])
            if phase >= 4:
                agg(2)
            if phase == 4:
                probe(o2tab[0][0:P, :])
            if phase >= 5:
                labels()

    nc.finalize()
    return nc


# ------------------------------------------------------------------ driver


def make_in_maps(cfg, prep, W1, b1, W2, b2, lin_W, lin_b):
    wv = lin_W.astype(np.float32).sum(axis=1)
    lay = prep["layout"]
    consts = dict(
        iota=prep["iota_rep"],
        iotac=prep["iota_col"],
        w1=W1.astype(np.float32).astype(ml_dtypes.bfloat16),
        w2=W2.astype(np.float32).astype(ml_dtypes.bfloat16),
        b1c=b1.astype(np.float32).reshape(P, 1),
        b2bc=np.tile(b2.astype(np.float32)[None, :], (P, 1)),
        wvrep=np.tile(wv[None, :], (P, lay["LABMAX"])),
    )
    in_maps = []
    for q in range(NC):
        m = dict(consts)
        m.update(
            xTq=prep["xT_shards"][q],
            gidx=prep["gidx"][q],
            mdst=prep["mdst"][q],
            mnrm=prep["mnrm"][q],
            mself=prep["mself"][q],
            lidx=prep["lidx"][q],
        )
        in_maps.append(m)
    return in_maps


def assemble_output(cfg, prep, results):
    out = np.zeros(cfg.n_labels, np.float32)
    order_arr = prep["order_arr"]
    for q in range(NC):
        r = np.asarray(results[q]["res"], np.float32)
        v = r.T.reshape(-1)
        m = order_arr[q] >= 0
        out[order_arr[q][m]] = v[m]
    return out


def run(cfg, x, edge_index, edge_weight, edge_label_index,
        W1, b1, W2, b2, lin_W, lin_b, trace=False, phase=99):
    global LAST_EXEC_NS, LAST_RESULTS
    prep = preprocess(cfg, np.asarray(x), np.asarray(edge_index),
                      np.asarray(edge_weight), np.asarray(edge_label_index))
    linb_sum = float(np.asarray(lin_b, np.float64).sum())
    nc = build_program(cfg, prep["layout"], linb_sum, phase=phase)
    in_maps = make_in_maps(cfg, prep, W1, b1, W2, b2, lin_W, lin_b)
    res = run_bass_kernel_spmd(
        nc, in_maps, list(range(NC)), trace=trace
    )
    LAST_EXEC_NS = res.exec_time_ns
    LAST_RESULTS = res
    return assemble_output(cfg, prep, res.results)


def kernel(x, edge_index, edge_weight, edge_label_index,
           W1, b1, W2, b2, lin_W, lin_b):
    trace = bool(os.environ.get("KERNEL_TRACE"))
    return run(FULL, x, edge_index, edge_weight, edge_label_index,
               W1, b1, W2, b2, lin_W, lin_b, trace=trace)


# revision 6
# speedup vs baseline: 1.4089x; 1.1742x over previous
"""GCN link-predictor kernel for 8 Trainium2 NeuronCores (Bass/Tile).

v2 — gather-wall-optimized SPMD design (single program, 8 cores):

  Per-core SWDGE dma_gather sustains ~111 GB/s (4 queues x ~30 GB/s,
  descriptor-dispatch-bound at 256B rows), so the kernel is organized to
  (a) minimize gathered rows and (b) keep everything else off the
  critical path:

  - Nodes: core q owns orig rows [q*12500,(q+1)*12500), padded to 12800
    (100 tiles of 128).  The global padded table is QUARTER-INTERLEAVED:
    node (q, quarter k, row r) lives at bank k (25600 rows < 2^15, int16
    gather indices), position q*3200 + r.  Each bank is produced by its
    own AllGather so gathers pipeline behind the collectives.
  - GEMM1 is sharded: each core computes h1 = x_shard @ W1 only for its
    own 12800 rows (vs full-table replicated in v1), stages to DRAM and
    AllGathers per quarter.
  - Aggregation (per layer): edges grouped per (dst-group of 5 tiles,
    src bank).  Within a (g,b) stream, slots are laid out per tile with
    uniform cross-core offsets (len = max over cores), padded only at
    stream ends -> ~7% padding (vs ~28% for per-(tile,bank) chunks).
    One-hot W columns are built per (chunk, tile) with norm-masked
    entries (pad/foreign slots have norm 0), batched in 2 DVE ops per
    (g,b).  PSUM accumulates K matmuls per dst tile; self-loops use
    SBUF-resident own-shard h tiles (zero gather descriptors).
  - Layer-2 GEMM is fused: the relu'd layer-1 psum tile (feature-major)
    is fed straight into one matmul with W2 producing the node-major h2
    tile; h2 is staged + AllGathered per quarter while agg1 continues.
  - Labels: 16 streams by (bank(a), bank(b)); both rows gathered
    node-major, product + lin_W-row-sum reduce on DVE.
"""

import os
import sys

import numpy as np

for _p in ("/opt/trn_rl_repo",):
    if _p not in sys.path:
        sys.path.insert(0, _p)

import ml_dtypes  # noqa: E402

import concourse.bacc as bacc  # noqa: E402
import concourse.bass as bass  # noqa: E402
import concourse.mybir as mybir  # noqa: E402
from concourse.bass_utils import run_bass_kernel_spmd  # noqa: E402
from concourse.library_config import mlp  # noqa: E402
from concourse.tile import TileContext  # noqa: E402
from concourse import dve_ops as _dvo  # noqa: E402
from concourse import dve_spec as _dvs  # noqa: E402
from concourse.dve_uop import DveOpSpec as _DveOpSpec  # noqa: E402


def _np_w_onehot_ref(in0, in1, s0, s1, imm2):
    x = in0.astype(np.float32)
    Sn, N = x.shape[1], x.shape[2]
    ee = (np.arange(Sn * N) - np.repeat(np.arange(Sn), N) * s1
          - s0).reshape(1, Sn, N)
    dd = x - ee
    return (np.maximum(dd, 0) * (dd < 1.0)).astype(np.float32)


def _register_w_onehot():
    for op in _dvo.OPS:
        if op.name == "W_ONEHOT_GCN":
            return op
    e = _dvs.Idx - _dvs.PageIdx(_dvs.C0, _dvs.C1)
    d = _dvs.Src0 - e
    spec = _dvs.Spec(body=_dvs.relu(d) * (d < _dvs.One),
                     reference=_np_w_onehot_ref)
    shas = {}
    for ver in ("v3", "v4"):
        uops = _dvs.lower(spec, ver=ver)
        shas[ver] = _DveOpSpec(
            name="W_ONEHOT_GCN", uops=uops, opcode=0,
            rd1_en=_dvo.has_src1(spec),
        ).sha(ver)
    op = _dvo.DveOp("W_ONEHOT_GCN", spec, subdim=True, uops_sha=shas)
    _dvo.OPS.append(op)
    _dvo.CUSTOM_DVE_SPECS[op.name] = spec
    _dvo._SUB_OPCODE_FOR_NAME[op.name] = (
        _dvo._CUSTOM_DVE_ROW_BASE + len(_dvo.OPS) - 1
    )
    return op


W_ONEHOT = _register_w_onehot()

P = 128
NC = 8
NBANK = 4
BF = mybir.dt.bfloat16
F32 = mybir.dt.float32
I16 = mybir.dt.int16

LAST_EXEC_NS = None
LAST_RESULTS = None

MAXCH = 8
GMAX = 4  # max dst tiles per aggregation group (one PSUM bank)


class Cfg:
    def __init__(self, n_nodes, n_labels):
        assert n_nodes % NC == 0
        self.n_nodes = n_nodes
        self.nodes_per_core = n_nodes // NC
        # pad shard to a multiple of 4 quarters of whole tiles
        self.tiles_per_core = -(-self.nodes_per_core // (4 * P)) * 4
        self.n_loc = self.tiles_per_core * P
        self.n_pad = NC * self.n_loc
        self.qrows = self.n_loc // 4          # shard rows per quarter
        self.bank_rows = NC * self.qrows      # rows per bank (= quarter)
        assert self.bank_rows < (1 << 15)
        self.qtiles = self.tiles_per_core // 4
        self.n_labels = n_labels
        self.lab_per_core = -(-n_labels // NC)


FULL = Cfg(100000, 200000)


def _groups_of(cfg):
    """Group sizes per quarter (each <= GMAX), tiled over 4 quarters."""
    qt = cfg.qtiles
    sizes = []
    r = qt
    while r > 0:
        s = min(GMAX, r)
        sizes.append(s)
        r -= s
    groups = []  # list of (tile0, ntiles) for the WHOLE shard
    for k in range(4):
        t0 = k * qt
        for s in sizes:
            groups.append((t0, s))
            t0 += s
    return groups


# ---------------------------------------------------------------- host prep


def _place(cfg, ids):
    """orig node id -> (core, shard_row, bank, bank_row)."""
    q = np.minimum(ids // cfg.nodes_per_core, NC - 1)
    r = ids - q * cfg.nodes_per_core
    k = r // cfg.qrows
    br = q * cfg.qrows + (r - k * cfg.qrows)
    return q, r, k, br


def preprocess(cfg, x, edge_index, edge_weight, edge_label_index):
    n = cfg.n_nodes
    T = cfg.tiles_per_core
    groups = _groups_of(cfg)
    NG = len(groups)

    src = edge_index[0].astype(np.int64)
    dst = edge_index[1].astype(np.int64)
    ew = edge_weight.astype(np.float32)
    # symmetric GCN normalization incl self loops (host scalar prep)
    deg = np.bincount(dst, weights=ew, minlength=n).astype(np.float32)
    deg += 1.0  # self loop weight
    dinv = (1.0 / np.sqrt(np.maximum(deg, 1e-12))).astype(np.float32)
    norm = dinv[src] * ew * dinv[dst]

    sq, sr, sk, sbr = _place(cfg, src)
    dq, dr, _, _ = _place(cfg, dst)
    d_tile = dr // P
    d_loc = dr % P

    # group id of each dst tile
    tile2g = np.zeros(T, np.int64)
    for gi, (t0, s) in enumerate(groups):
        tile2g[t0: t0 + s] = gi
    d_g = tile2g[d_tile]

    # ---- per (core, group, bank, tile) counts -> uniform slot layout ----
    NKEY = NG * NBANK * T
    key_t = (d_g * NBANK + sk) * T + d_tile
    key_full = (dq * NG * NBANK + d_g * NBANK + sk) * T + d_tile
    cnt = np.bincount(key_full, minlength=NC * NKEY).reshape(NC, NG, NBANK, T)
    tlen = cnt.max(axis=0)  # [NG, NBANK, T] uniform per-tile slot lengths

    # stream/segment layout (identical across cores)
    #  per (g,b): tiles t0..t0+s-1 at offsets off[t], total padded to 128
    nch = np.zeros((NG, NBANK), np.int64)      # chunks per (g,b)
    toff = np.zeros((NG, NBANK, T), np.int64)  # slot offset of tile in stream
    c0 = np.zeros((NG, NBANK), np.int64)       # first chunk col of (g,b)
    segs = [[None] * NBANK for _ in range(NG)]  # per (g,b): list of seg dicts
    ct0 = 0
    CT_cols = []   # flat mdst/mnrm segment column count
    cseq = 0
    for gi, (t0, s) in enumerate(groups):
        for b in range(NBANK):
            off = 0
            sl = []
            for t in range(t0, t0 + s):
                toff[gi, b, t] = off
                off += int(tlen[gi, b, t])
            tot = off
            ch = -(-tot // P) if tot else 0
            nch[gi, b] = ch
            c0[gi, b] = cseq
            cseq += ch
            # segments: (tile, chunk, ct_col)
            for t in range(t0, t0 + s):
                lo, hi = int(toff[gi, b, t]), int(toff[gi, b, t] + tlen[gi, b, t])
                if hi == lo:
                    continue
                for c in range(lo // P, -(-hi // P)):
                    sl.append((t, c, ct0))
                    ct0 += 1
            segs[gi][b] = sl
    CE = cseq          # total edge chunks per core
    CT = ct0           # total masked meta columns
    NCTMAX = max(
        (len(segs[gi][b]) for gi in range(NG) for b in range(NBANK)),
        default=1,
    )
    NCHMAX = int(nch.max())

    # ---- place edges into slots ----
    order = np.lexsort((src, key_t, dq))  # by (core, g, b, tile), stable
    so = order
    core_o = dq[so]
    g_o = d_g[so]
    b_o = sk[so]
    t_o = d_tile[so]
    # position within (core,g,b,t)
    sk2 = (core_o * NG * NBANK + g_o * NBANK + b_o) * T + t_o
    starts = np.zeros(NC * NKEY + 1, np.int64)
    starts[1:] = np.cumsum(cnt.reshape(-1))
    pos_in = np.arange(len(so)) - starts[sk2]
    slot = toff[g_o, b_o, t_o] + pos_in            # slot within (g,b) stream
    chunk = c0[g_o, b_o] + slot // P               # global chunk col
    part = slot % P

    gidx16 = np.zeros((NC, 16, CE * 8), np.int16)
    gidx16[core_o, part % 16, chunk * 8 + part // 16] = sbr[so].astype(np.int16)
    gidx = np.ascontiguousarray(np.tile(gidx16, (1, 8, 1)))

    # masked per-(chunk,tile) meta columns, packed v = dstl + norm (f32)
    mv_a = np.zeros((NC, P, CT), np.float32)
    # vectorized edge -> ct col: ct = ct_base[g,b,t] + slot//P - clo[g,b,t]
    ct_base = np.zeros((NG, NBANK, T), np.int64)
    clo_arr = np.zeros((NG, NBANK, T), np.int64)
    for gi in range(NG):
        for b in range(NBANK):
            for (t, c, ct) in segs[gi][b]:
                if ct_base[gi, b, t] == 0 and clo_arr[gi, b, t] == 0:
                    ct_base[gi, b, t] = ct
                    clo_arr[gi, b, t] = c
                else:
                    ct_base[gi, b, t] = min(ct_base[gi, b, t], ct)
                    clo_arr[gi, b, t] = min(clo_arr[gi, b, t], c)
    ct_of = (ct_base[g_o, b_o, t_o] + slot // P - clo_arr[g_o, b_o, t_o])
    mv_a[core_o, part, ct_of] = d_loc[so] + norm[so]

    # per-tile self-loop scale dinv^2 (0 on padded rows)
    mself_a = np.zeros((NC, P, T), np.float32)
    for q in range(NC):
        lo = q * cfg.nodes_per_core
        cnt_q = cfg.nodes_per_core
        d2 = dinv[lo: lo + cnt_q] ** 2
        full = np.zeros(cfg.n_loc, np.float32)
        full[:cnt_q] = d2
        mself_a[q] = full.reshape(T, P).T
    mself = mself_a.astype(ml_dtypes.bfloat16)

    # ---- labels: 16 streams by (bank(a), bank(b)) ----
    ela = edge_label_index[0].astype(np.int64)
    elb = edge_label_index[1].astype(np.int64)
    _, _, ak, abr = _place(cfg, ela)
    _, _, bk, bbr = _place(cfg, elb)
    bp_all = ak * NBANK + bk
    lpc = cfg.lab_per_core
    NBP = NBANK * NBANK
    cnts = np.zeros((NC, NBP), np.int64)
    for q in range(NC):
        lo, hi = q * lpc, min((q + 1) * lpc, cfg.n_labels)
        cnts[q] = np.bincount(bp_all[lo:hi], minlength=NBP)
    lkb = (-(-cnts.max(axis=0) // P)).astype(np.int64)
    lchunk0 = np.zeros(NBP + 1, np.int64)
    lchunk0[1:] = np.cumsum(lkb)
    LCp = int(lchunk0[-1])
    LABMAX = int(lkb.max())
    lcol0a = lchunk0[:-1] * 16
    lcol0b = lcol0a + lkb * 8
    LICOLS = LCp * 16

    lidx16 = np.zeros((NC, 16, LICOLS), np.int16)
    order_arr = np.full((NC, LCp * P), -1, np.int64)
    for q in range(NC):
        lo, hi = q * lpc, min((q + 1) * lpc, cfg.n_labels)
        bp_q = bp_all[lo:hi]
        oq = np.argsort(bp_q, kind="stable")
        sbp = bp_q[oq]
        st = np.zeros(NBP + 1, np.int64)
        st[1:] = np.cumsum(cnts[q])
        pos = np.arange(len(oq)) - st[sbp]
        cola = lcol0a[sbp] + pos // 16
        colb = lcol0b[sbp] + pos // 16
        prt = pos % 16
        lidx16[q, prt, cola] = abr[lo:hi][oq].astype(np.int16)
        lidx16[q, prt, colb] = bbr[lo:hi][oq].astype(np.int16)
        sl = (lchunk0[sbp] + pos // P) * P + pos % P
        order_arr[q, sl] = lo + oq
    lidx = np.ascontiguousarray(np.tile(lidx16, (1, 8, 1)))

    # ---- node features shards, feature-major ----
    xT_shards = []
    for q in range(NC):
        lo = q * cfg.nodes_per_core
        xs = np.zeros((cfg.n_loc, P), np.float32)
        xs[: cfg.nodes_per_core] = x[lo: lo + cfg.nodes_per_core]
        xT_shards.append(
            np.ascontiguousarray(xs.T).astype(ml_dtypes.bfloat16)
        )

    iota_rep = np.tile(
        np.arange(P, dtype=np.float32)[None, :], (P, max(NCTMAX, 1))
    ).astype(ml_dtypes.bfloat16)
    iota_col = np.arange(P, dtype=np.float32).reshape(P, 1).astype(
        ml_dtypes.bfloat16
    )

    layout = dict(
        groups=groups, nch=nch, c0=c0, segs=segs, CE=CE, CT=CT,
        NCTMAX=NCTMAX, NCHMAX=NCHMAX,
        lkb=[int(v) for v in lkb], lchunk0=[int(v) for v in lchunk0],
        LCp=LCp, LABMAX=LABMAX,
        lcol0a=[int(v) for v in lcol0a], lcol0b=[int(v) for v in lcol0b],
        LICOLS=LICOLS,
    )
    return dict(gidx=gidx, mv=mv_a, mself=mself, lidx=lidx,
                xT_shards=xT_shards, iota_rep=iota_rep, iota_col=iota_col,
                order_arr=order_arr, layout=layout)


# ------------------------------------------------------------- bass program


def build_program(cfg, lay, linb_sum, phase=99):
    T = cfg.tiles_per_core
    QR = cfg.qrows
    BR = cfg.bank_rows
    QT = cfg.qtiles
    groups = lay["groups"]
    NG = len(groups)
    nch, c0g, segs = lay["nch"], lay["c0"], lay["segs"]
    CE, CT, NCTMAX, NCHMAX = lay["CE"], lay["CT"], lay["NCTMAX"], lay["NCHMAX"]
    LCp, LABMAX = lay["LCp"], lay["LABMAX"]
    lkb, lchunk0 = lay["lkb"], lay["lchunk0"]
    lcol0a, lcol0b = lay["lcol0a"], lay["lcol0b"]
    rg = [list(range(NC))]

    nc = bacc.Bacc(None, target_bir_lowering=False, debug=False,
                   dynamic_dma_scratch_size=16384, num_swdge_queues=4)
    qrr = [0]

    def next_q():
        qrr[0] = (qrr[0] + 1) % 4
        return qrr[0]

    xT_d = nc.declare_dram_parameter("xTq", [P, cfg.n_loc], BF, False)
    gidx_d = nc.declare_dram_parameter("gidx", [P, CE * 8], I16, False)
    mv_d = nc.declare_dram_parameter("mv", [P, CT], F32, False)
    mself_d = nc.declare_dram_parameter("mself", [P, T], BF, False)
    iota_d = nc.declare_dram_parameter("iota", [P, NCTMAX * P], BF, False)
    iotac_d = nc.declare_dram_parameter("iotac", [P, 1], BF, False)
    lidx_d = nc.declare_dram_parameter("lidx", [P, lay["LICOLS"]], I16, False)
    w1_d = nc.declare_dram_parameter("w1", [P, P], BF, False)
    w2_d = nc.declare_dram_parameter("w2", [P, P], BF, False)
    b1c_d = nc.declare_dram_parameter("b1c", [P, 1], F32, False)
    b2_d = nc.declare_dram_parameter("b2bc", [P, P], F32, False)
    wv_d = nc.declare_dram_parameter("wvrep", [P, LABMAX * P], F32, False)
    res_d = nc.declare_dram_parameter("res", [P, LCp], F32, True)

    h1_sh = [nc.dram_tensor(f"h1sh{k}", [QR, P], BF) for k in range(4)]
    h2_sh = [nc.dram_tensor(f"h2sh{k}", [QR, P], BF) for k in range(4)]
    o2_sh = [nc.dram_tensor(f"o2sh{k}", [QR, P], BF) for k in range(4)]
    htab1 = [
        nc.dram_tensor(f"htab1_{k}", [BR, P], BF, addr_space="Shared")
        for k in range(4)
    ]
    htab2 = [
        nc.dram_tensor(f"htab2_{k}", [BR, P], BF, addr_space="Shared")
        for k in range(4)
    ]
    o2tab = [
        nc.dram_tensor(f"o2tab_{k}", [BR, P], BF, addr_space="Shared")
        for k in range(4)
    ]

    AF = mybir.ActivationFunctionType
    OP = mybir.AluOpType

    with TileContext(nc) as tc:
        with (
            tc.tile_pool(name="const", bufs=1) as cp,
            tc.tile_pool(name="xload", bufs=2) as xp,
            tc.tile_pool(name="gemmev", bufs=3) as gep,
            tc.tile_pool(name="gitile", bufs=2) as gip,
            tc.tile_pool(name="hgb", bufs=6) as hp,
            tc.tile_pool(name="wgb", bufs=4) as wp,
            tc.tile_pool(name="aggev", bufs=3) as aep,
            tc.tile_pool(name="lab", bufs=2) as lp,
            tc.tile_pool(name="ps_gemm", bufs=2, space="PSUM") as psg,
            tc.tile_pool(name="ps_agg", bufs=3, space="PSUM") as psa,
        ):
            nc.gpsimd.load_library(mlp)
            # ---- persistent SBUF ----
            mv_sb = cp.tile([P, CT], F32)
            nc.sync.dma_start(out=mv_sb[:], in_=mv_d[:, :])
            mself_sb = cp.tile([P, T], BF)
            nc.sync.dma_start(out=mself_sb[:], in_=mself_d[:, :])
            iota_sb = cp.tile([P, NCTMAX * P], BF)
            nc.sync.dma_start(out=iota_sb[:], in_=iota_d[:, :])
            iotac_sb = cp.tile([P, 1], BF)
            nc.sync.dma_start(out=iotac_sb[:], in_=iotac_d[:, :])
            lidx_sb = cp.tile([P, lay["LICOLS"]], I16)
            nc.sync.dma_start(out=lidx_sb[:], in_=lidx_d[:, :])
            w1_sb = cp.tile([P, P], BF)
            nc.sync.dma_start(out=w1_sb[:], in_=w1_d[:, :])
            w2_sb = cp.tile([P, P], BF)
            nc.sync.dma_start(out=w2_sb[:], in_=w2_d[:, :])
            b1c_sb = cp.tile([P, 1], F32)
            nc.sync.dma_start(out=b1c_sb[:], in_=b1c_d[:, :])
            b2_sb = cp.tile([P, P], F32)
            nc.sync.dma_start(out=b2_sb[:], in_=b2_d[:, :])
            wv_sb = cp.tile([P, LABMAX * P], F32)
            nc.sync.dma_start(out=wv_sb[:], in_=wv_d[:, :])
            res_sb = cp.tile([P, LCp], F32)

            selfh1 = cp.tile([P, T * P], BF)   # dinv^2-scaled own h1 tiles
            selfh2 = cp.tile([P, T * P], BF)
            # identity one-hot for self chunks
            idmat = cp.tile([P, P], BF)
            nc.vector.tensor_tensor(
                out=idmat[:],
                in0=iota_sb[:, :P],
                in1=iotac_sb[:].to_broadcast([P, P]),
                op=OP.is_equal,
            )

            iota3 = iota_sb[:].rearrange("p (g e) -> p g e", e=P)

            # ---- sharded GEMM1 + quarter AllGathers ----
            def gemm1():
                G1 = 4
                for k in range(4):
                    for t0 in range(k * QT, (k + 1) * QT, G1):
                        gs = min(G1, (k + 1) * QT - t0)
                        lhsT = xp.tile([P, 4 * P], BF, tag="x")
                        nc.scalar.dma_start(
                            out=lhsT[:, : gs * P],
                            in_=xT_d[:, t0 * P: (t0 + gs) * P],
                        )
                        pg = psg.tile([P, 4 * P], F32, tag="g1")
                        for i in range(gs):
                            nc.tensor.matmul(
                                out=pg[:, i * P: (i + 1) * P],
                                lhsT=lhsT[:, i * P: (i + 1) * P],
                                rhs=w1_sb[:],
                                start=True, stop=True,
                            )
                        hb = gep.tile([P, 4 * P], BF, tag="hb")
                        nc.scalar.activation(
                            hb[:, : gs * P], pg[:, : gs * P], AF.Copy
                        )
                        # self-scaled copy (dinv^2 per node row)
                        nc.vector.tensor_tensor(
                            out=selfh1[:, t0 * P: (t0 + gs) * P]
                            .rearrange("p (g e) -> p g e", e=P),
                            in0=hb[:, : gs * P]
                            .rearrange("p (g e) -> p g e", e=P),
                            in1=mself_sb[:, t0: t0 + gs]
                            .to_broadcast([P, gs, P]),
                            op=OP.mult,
                        )
                        nc.sync.dma_start(
                            out=h1_sh[k][
                                (t0 - k * QT) * P: (t0 - k * QT + gs) * P, :
                            ].rearrange("(i p) j -> p i j", p=P),
                            in_=hb[:, : gs * P]
                            .rearrange("p (i j) -> p i j", j=P),
                        )
                    nc.gpsimd.collective_compute(
                        "AllGather", OP.bypass, replica_groups=rg,
                        ins=[h1_sh[k][:, :]], outs=[htab1[k][:, :]],
                    )

            # ---- aggregation layer (fused gemm2 after layer 1) ----
            def agg(layer):
                htab = htab1 if layer == 1 else htab2
                sh = selfh1 if layer == 1 else selfh2
                for gi, (t0, gs) in enumerate(groups):
                    chg = int(nch[gi].sum())
                    if chg:
                        gt = gip.tile([P, NCHMAX * NBANK * 8], I16, tag="gi")
                        nc.sync.dma_start(
                            out=gt[:, : chg * 8],
                            in_=gidx_d[
                                :, c0g[gi, 0] * 8: (c0g[gi, 0] + chg) * 8
                            ],
                        )
                    # gathers per bank
                    htiles = {}
                    for b in range(NBANK):
                        ch = int(nch[gi, b])
                        if ch == 0:
                            continue
                        h = hp.tile([P, NCHMAX * P], BF, tag="h")
                        htiles[b] = h
                        gtoff = (c0g[gi, b] - c0g[gi, 0]) * 8
                        for cc in range(0, ch, MAXCH):
                            cw = min(MAXCH, ch - cc)
                            nc.gpsimd.dma_gather(
                                h[:, cc * P: (cc + cw) * P].rearrange(
                                    "p (c e) -> p c e", e=P
                                ),
                                htab[b][:, :],
                                gt[:, gtoff + cc * 8: gtoff + (cc + cw) * 8],
                                cw * P, cw * P, P,
                                queue_num=next_q(),
                            )
                    # build all W tiles for the group first (DVE)
                    wtiles = {}
                    ct_firsts = {}
                    for b in range(NBANK):
                        sl = segs[gi][b]
                        if not sl:
                            continue
                        nct = len(sl)
                        ct_first = sl[0][2]
                        ct_firsts[b] = ct_first
                        w = wp.tile([P, NCTMAX * P], BF, tag="w")
                        wtiles[b] = w
                        w3 = w[:, : nct * P].rearrange(
                            "p (g e) -> p g e", e=P
                        )
                        nc.vector._custom_dve(
                            W_ONEHOT,
                            out=w3,
                            in0=mv_sb[:, ct_first: ct_first + nct]
                            .to_broadcast([P, nct, P]),
                            s0=0.0, s1=float(P),
                        )
                    # per-tile segment lists: a psum region's accumulation
                    # must run start->stop contiguously (interleaved open
                    # accumulations within one bank corrupt results)
                    tsegs = {t: [] for t in range(t0, t0 + gs)}
                    for b in range(NBANK):
                        if segs[gi][b]:
                            for (t, c, ct) in segs[gi][b]:
                                tsegs[t].append((b, c, ct))
                    pgrp = psa.tile([P, GMAX * P], F32)
                    pts = {}
                    for i, t in enumerate(range(t0, t0 + gs)):
                        pts[t] = pgrp[:, i * P: (i + 1) * P]
                        shs = sh[:, t * P: (t + 1) * P]
                        onlyself = not tsegs[t]
                        if layer == 1:
                            nc.tensor.matmul(
                                out=pts[t], lhsT=shs, rhs=idmat[:],
                                start=True, stop=onlyself,
                            )
                        else:
                            nc.tensor.matmul(
                                out=pts[t], lhsT=idmat[:], rhs=shs,
                                start=True, stop=onlyself,
                            )
                        nseg = len(tsegs[t])
                        for si, (b, c, ct) in enumerate(tsegs[t]):
                            hs = htiles[b][:, c * P: (c + 1) * P]
                            ws = wtiles[b][
                                :, (ct - ct_firsts[b]) * P:
                                (ct - ct_firsts[b] + 1) * P]
                            stop = si == nseg - 1
                            if layer == 1:
                                nc.tensor.matmul(
                                    out=pts[t], lhsT=hs, rhs=ws,
                                    start=False, stop=stop,
                                )
                            else:
                                nc.tensor.matmul(
                                    out=pts[t], lhsT=ws, rhs=hs,
                                    start=False, stop=stop,
                                )
                    # evictions
                    k = t0 // QT
                    tq0 = t0 - k * QT
                    if layer == 1:
                        # relu(psum + b1) feature-major -> fused gemm2 ->
                        # node-major h2 tile
                        ob = aep.tile([P, GMAX * P], BF, tag="ob")
                        h2b = aep.tile([P, GMAX * P], BF, tag="h2b")
                        for i, t in enumerate(range(t0, t0 + gs)):
                            nc.scalar.activation(
                                ob[:, i * P: (i + 1) * P], pts[t],
                                AF.Relu, bias=b1c_sb[:],
                            )
                        for i, t in enumerate(range(t0, t0 + gs)):
                            pg2 = psg.tile([P, P], F32, tag="g2")
                            nc.tensor.matmul(
                                out=pg2[:],
                                lhsT=ob[:, i * P: (i + 1) * P],
                                rhs=w2_sb[:],
                                start=True, stop=True,
                            )
                            nc.scalar.activation(
                                h2b[:, i * P: (i + 1) * P], pg2[:], AF.Copy
                            )
                        nc.vector.tensor_tensor(
                            out=selfh2[:, t0 * P: (t0 + gs) * P]
                            .rearrange("p (g e) -> p g e", e=P),
                            in0=h2b[:, : gs * P]
                            .rearrange("p (g e) -> p g e", e=P),
                            in1=mself_sb[:, t0: t0 + gs]
                            .to_broadcast([P, gs, P]),
                            op=OP.mult,
                        )
                        nc.sync.dma_start(
                            out=h2_sh[k][tq0 * P: (tq0 + gs) * P, :]
                            .rearrange("(i p) j -> p i j", p=P),
                            in_=h2b[:, : gs * P]
                            .rearrange("p (i j) -> p i j", j=P),
                        )
                    else:
                        # node-major psum: +b2 (DVE), relu (scalar)
                        o2b = aep.tile([P, GMAX * P], BF, tag="o2b")
                        for i, t in enumerate(range(t0, t0 + gs)):
                            t1 = aep.tile([P, P], F32, tag="t1")
                            nc.vector.tensor_tensor(
                                out=t1[:], in0=pts[t], in1=b2_sb[:],
                                op=OP.add,
                            )
                            nc.scalar.activation(
                                o2b[:, i * P: (i + 1) * P], t1[:], AF.Relu
                            )
                        nc.sync.dma_start(
                            out=o2_sh[k][tq0 * P: (tq0 + gs) * P, :]
                            .rearrange("(i p) j -> p i j", p=P),
                            in_=o2b[:, : gs * P]
                            .rearrange("p (i j) -> p i j", j=P),
                        )
                    if tq0 + gs == QT:  # quarter complete
                        if layer == 1:
                            nc.gpsimd.collective_compute(
                                "AllGather", OP.bypass, replica_groups=rg,
                                ins=[h2_sh[k][:, :]], outs=[htab2[k][:, :]],
                            )
                        else:
                            nc.gpsimd.collective_compute(
                                "AllGather", OP.bypass, replica_groups=rg,
                                ins=[o2_sh[k][:, :]], outs=[o2tab[k][:, :]],
                            )

            def labels():
                NBP = NBANK * NBANK
                bporder = sorted(range(NBP), key=lambda bp: max(
                    bp // NBANK, bp % NBANK))
                for bp in bporder:
                    nchb = lkb[bp]
                    if nchb == 0:
                        continue
                    b0, b1 = divmod(bp, NBANK)
                    a = lp.tile([P, LABMAX * P], BF, tag="a")
                    bb = lp.tile([P, LABMAX * P], BF, tag="b")
                    for tile_, bank, col0 in (
                        (a, b0, lcol0a[bp]),
                        (bb, b1, lcol0b[bp]),
                    ):
                        for cc in range(0, nchb, MAXCH):
                            cw = min(MAXCH, nchb - cc)
                            nc.gpsimd.dma_gather(
                                tile_[:, cc * P: (cc + cw) * P].rearrange(
                                    "p (c e) -> p c e", e=P
                                ),
                                o2tab[bank][:, :],
                                lidx_sb[:, col0 + cc * 8: col0 + (cc + cw) * 8],
                                cw * P, cw * P, P,
                                queue_num=next_q(),
                            )
                    prod = lp.tile([P, LABMAX * P], F32, tag="prod")
                    nc.vector.tensor_tensor(
                        out=prod[:, : nchb * P],
                        in0=a[:, : nchb * P],
                        in1=bb[:, : nchb * P],
                        op=OP.mult,
                    )
                    nc.vector.tensor_tensor(
                        out=prod[:, : nchb * P],
                        in0=prod[:, : nchb * P],
                        in1=wv_sb[:, : nchb * P],
                        op=OP.mult,
                    )
                    nc.vector.reduce_sum(
                        res_sb[:, lchunk0[bp]: lchunk0[bp] + nchb],
                        prod[:, : nchb * P].rearrange(
                            "p (g e) -> p g e", e=P
                        ),
                        axis=mybir.AxisListType.X,
                    )
                nc.vector.tensor_scalar_add(
                    res_sb[:], res_sb[:], float(linb_sum)
                )
                nc.sync.dma_start(out=res_d[:, :], in_=res_sb[:])

            def probe(src_ap, cast=True):
                prb = cp.tile([P, P], F32)
                if cast:
                    tmp = cp.tile([P, P], BF)
                    nc.sync.dma_start(out=tmp[:], in_=src_ap)
                    nc.vector.tensor_copy(prb[:], tmp[:])
                pb = min(LCp, P)
                nc.sync.dma_start(out=res_d[:, :pb], in_=prb[:, :pb])

            if phase >= 2:
                gemm1()
            if phase == 2:
                probe(htab1[0][0:P, :])
            if phase >= 3:
                agg(1)
            if phase == 3:
                probe(htab2[0][0:P, :])
            if phase >= 4:
                agg(2)
            if phase == 4:
                probe(o2tab[0][0:P, :])
            if phase >= 5:
                labels()

    nc.finalize()
    return nc


# ------------------------------------------------------------------ driver


def make_in_maps(cfg, prep, W1, b1, W2, b2, lin_W, lin_b):
    wv = lin_W.astype(np.float32).sum(axis=1)
    lay = prep["layout"]
    consts = dict(
        iota=prep["iota_rep"],
        iotac=prep["iota_col"],
        w1=W1.astype(np.float32).astype(ml_dtypes.bfloat16),
        w2=W2.astype(np.float32).astype(ml_dtypes.bfloat16),
        b1c=b1.astype(np.float32).reshape(P, 1),
        b2bc=np.tile(b2.astype(np.float32)[None, :], (P, 1)),
        wvrep=np.tile(wv[None, :], (P, lay["LABMAX"])),
    )
    in_maps = []
    for q in range(NC):
        m = dict(consts)
        m.update(
            xTq=prep["xT_shards"][q],
            gidx=prep["gidx"][q],
            mv=prep["mv"][q],
            mself=prep["mself"][q],
            lidx=prep["lidx"][q],
        )
        in_maps.append(m)
    return in_maps


def assemble_output(cfg, prep, results):
    out = np.zeros(cfg.n_labels, np.float32)
    order_arr = prep["order_arr"]
    for q in range(NC):
        r = np.asarray(results[q]["res"], np.float32)
        v = r.T.reshape(-1)
        m = order_arr[q] >= 0
        out[order_arr[q][m]] = v[m]
    return out


def run(cfg, x, edge_index, edge_weight, edge_label_index,
        W1, b1, W2, b2, lin_W, lin_b, trace=False, phase=99):
    global LAST_EXEC_NS, LAST_RESULTS
    prep = preprocess(cfg, np.asarray(x), np.asarray(edge_index),
                      np.asarray(edge_weight), np.asarray(edge_label_index))
    linb_sum = float(np.asarray(lin_b, np.float64).sum())
    nc = build_program(cfg, prep["layout"], linb_sum, phase=phase)
    in_maps = make_in_maps(cfg, prep, W1, b1, W2, b2, lin_W, lin_b)
    res = run_bass_kernel_spmd(
        nc, in_maps, list(range(NC)), trace=trace
    )
    LAST_EXEC_NS = res.exec_time_ns
    LAST_RESULTS = res
    return assemble_output(cfg, prep, res.results)


def kernel(x, edge_index, edge_weight, edge_label_index,
           W1, b1, W2, b2, lin_W, lin_b):
    trace = bool(os.environ.get("KERNEL_TRACE"))
    return run(FULL, x, edge_index, edge_weight, edge_label_index,
               W1, b1, W2, b2, lin_W, lin_b, trace=trace)


# revision 8
# speedup vs baseline: 1.5431x; 1.0952x over previous
"""GCN link-predictor kernel for 8 Trainium2 NeuronCores (Bass/Tile).

v2 — gather-wall-optimized SPMD design (single program, 8 cores):

  Per-core SWDGE dma_gather sustains ~111 GB/s (4 queues x ~30 GB/s,
  descriptor-dispatch-bound at 256B rows), so the kernel is organized to
  (a) minimize gathered rows and (b) keep everything else off the
  critical path:

  - Nodes: core q owns orig rows [q*12500,(q+1)*12500), padded to 12800
    (100 tiles of 128).  The global padded table is QUARTER-INTERLEAVED:
    node (q, quarter k, row r) lives at bank k (25600 rows < 2^15, int16
    gather indices), position q*3200 + r.  Each bank is produced by its
    own AllGather so gathers pipeline behind the collectives.
  - GEMM1 is sharded: each core computes h1 = x_shard @ W1 only for its
    own 12800 rows (vs full-table replicated in v1), stages to DRAM and
    AllGathers per quarter.
  - Aggregation (per layer): edges grouped per (dst-group of 5 tiles,
    src bank).  Within a (g,b) stream, slots are laid out per tile with
    uniform cross-core offsets (len = max over cores), padded only at
    stream ends -> ~7% padding (vs ~28% for per-(tile,bank) chunks).
    One-hot W columns are built per (chunk, tile) with norm-masked
    entries (pad/foreign slots have norm 0), batched in 2 DVE ops per
    (g,b).  PSUM accumulates K matmuls per dst tile; self-loops use
    SBUF-resident own-shard h tiles (zero gather descriptors).
  - Layer-2 GEMM is fused: the relu'd layer-1 psum tile (feature-major)
    is fed straight into one matmul with W2 producing the node-major h2
    tile; h2 is staged + AllGathered per quarter while agg1 continues.
  - Labels: 16 streams by (bank(a), bank(b)); both rows gathered
    node-major, product + lin_W-row-sum reduce on DVE.
"""

import os
import sys

import numpy as np

for _p in ("/opt/trn_rl_repo",):
    if _p not in sys.path:
        sys.path.insert(0, _p)

import ml_dtypes  # noqa: E402

import concourse.bacc as bacc  # noqa: E402
import concourse.bass as bass  # noqa: E402
import concourse.mybir as mybir  # noqa: E402
from concourse.bass_utils import run_bass_kernel_spmd  # noqa: E402
from concourse.library_config import mlp  # noqa: E402
from concourse.tile import TileContext  # noqa: E402
from concourse import dve_ops as _dvo  # noqa: E402
from concourse import dve_spec as _dvs  # noqa: E402
from concourse.dve_uop import DveOpSpec as _DveOpSpec  # noqa: E402


def _np_w_onehot_ref(in0, in1, s0, s1, imm2):
    x = in0.astype(np.float32)
    Sn, N = x.shape[1], x.shape[2]
    ee = (np.arange(Sn * N) - np.repeat(np.arange(Sn), N) * s1
          - s0).reshape(1, Sn, N)
    dd = x - ee
    return (np.maximum(dd, 0) * (dd < 1.0)).astype(np.float32)


def _register_w_onehot():
    for op in _dvo.OPS:
        if op.name == "W_ONEHOT_GCN":
            return op
    e = _dvs.Idx - _dvs.PageIdx(_dvs.C0, _dvs.C1)
    d = _dvs.Src0 - e
    spec = _dvs.Spec(body=_dvs.relu(d) * (d < _dvs.One),
                     reference=_np_w_onehot_ref)
    shas = {}
    for ver in ("v3", "v4"):
        uops = _dvs.lower(spec, ver=ver)
        shas[ver] = _DveOpSpec(
            name="W_ONEHOT_GCN", uops=uops, opcode=0,
            rd1_en=_dvo.has_src1(spec),
        ).sha(ver)
    op = _dvo.DveOp("W_ONEHOT_GCN", spec, subdim=True, uops_sha=shas)
    _dvo.OPS.append(op)
    _dvo.CUSTOM_DVE_SPECS[op.name] = spec
    _dvo._SUB_OPCODE_FOR_NAME[op.name] = (
        _dvo._CUSTOM_DVE_ROW_BASE + len(_dvo.OPS) - 1
    )
    return op


W_ONEHOT = _register_w_onehot()

P = 128
NC = 8
NBANK = 4
BF = mybir.dt.bfloat16
F32 = mybir.dt.float32
I16 = mybir.dt.int16

LAST_EXEC_NS = None
LAST_RESULTS = None

MAXCH = 8
GMAX = 4  # max dst tiles per aggregation group (one PSUM bank)


class Cfg:
    def __init__(self, n_nodes, n_labels):
        assert n_nodes % NC == 0
        self.n_nodes = n_nodes
        self.nodes_per_core = n_nodes // NC
        # pad shard to a multiple of 4 quarters of whole tiles
        self.tiles_per_core = -(-self.nodes_per_core // (4 * P)) * 4
        self.n_loc = self.tiles_per_core * P
        self.n_pad = NC * self.n_loc
        self.qrows = self.n_loc // 4          # shard rows per quarter
        self.bank_rows = NC * self.qrows      # rows per bank (= quarter)
        assert self.bank_rows < (1 << 15)
        self.qtiles = self.tiles_per_core // 4
        self.n_labels = n_labels
        self.lab_per_core = -(-n_labels // NC)


FULL = Cfg(100000, 200000)


def _groups_of(cfg):
    """Group sizes per quarter (each <= GMAX), tiled over 4 quarters."""
    qt = cfg.qtiles
    sizes = []
    r = qt
    while r > 0:
        s = min(GMAX, r)
        sizes.append(s)
        r -= s
    groups = []  # list of (tile0, ntiles) for the WHOLE shard
    for k in range(4):
        t0 = k * qt
        for s in sizes:
            groups.append((t0, s))
            t0 += s
    return groups


# ---------------------------------------------------------------- host prep


def _place(cfg, ids):
    """orig node id -> (core, shard_row, bank, bank_row)."""
    q = np.minimum(ids // cfg.nodes_per_core, NC - 1)
    r = ids - q * cfg.nodes_per_core
    k = r // cfg.qrows
    br = q * cfg.qrows + (r - k * cfg.qrows)
    return q, r, k, br


def preprocess(cfg, x, edge_index, edge_weight, edge_label_index):
    n = cfg.n_nodes
    T = cfg.tiles_per_core
    groups = _groups_of(cfg)
    NG = len(groups)

    src = edge_index[0].astype(np.int64)
    dst = edge_index[1].astype(np.int64)
    ew = edge_weight.astype(np.float32)
    # symmetric GCN normalization incl self loops (host scalar prep)
    deg = np.bincount(dst, weights=ew, minlength=n).astype(np.float32)
    deg += 1.0  # self loop weight
    dinv = (1.0 / np.sqrt(np.maximum(deg, 1e-12))).astype(np.float32)
    norm = dinv[src] * ew * dinv[dst]

    sq, sr, sk, sbr = _place(cfg, src)
    dq, dr, _, _ = _place(cfg, dst)
    d_tile = dr // P
    d_loc = dr % P

    # group id of each dst tile
    tile2g = np.zeros(T, np.int64)
    for gi, (t0, s) in enumerate(groups):
        tile2g[t0: t0 + s] = gi
    d_g = tile2g[d_tile]

    # ---- per (core, group, bank, tile) counts -> uniform slot layout ----
    NKEY = NG * NBANK * T
    key_t = (d_g * NBANK + sk) * T + d_tile
    key_full = (dq * NG * NBANK + d_g * NBANK + sk) * T + d_tile
    cnt = np.bincount(key_full, minlength=NC * NKEY).reshape(NC, NG, NBANK, T)
    tlen = cnt.max(axis=0)  # [NG, NBANK, T] uniform per-tile slot lengths

    # stream/segment layout (identical across cores)
    #  per (g,b): tiles t0..t0+s-1 at offsets off[t], total padded to 128
    nch = np.zeros((NG, NBANK), np.int64)      # chunks per (g,b)
    toff = np.zeros((NG, NBANK, T), np.int64)  # slot offset of tile in stream
    c0 = np.zeros((NG, NBANK), np.int64)       # first chunk col of (g,b)
    segs = [[None] * NBANK for _ in range(NG)]  # per (g,b): list of seg dicts
    ct0 = 0
    CT_cols = []   # flat mdst/mnrm segment column count
    cseq = 0
    for gi, (t0, s) in enumerate(groups):
        for b in range(NBANK):
            off = 0
            sl = []
            for t in range(t0, t0 + s):
                toff[gi, b, t] = off
                off += int(tlen[gi, b, t])
            tot = off
            ch = -(-tot // P) if tot else 0
            nch[gi, b] = ch
            c0[gi, b] = cseq
            cseq += ch
            # segments: (tile, chunk, ct_col)
            for t in range(t0, t0 + s):
                lo, hi = int(toff[gi, b, t]), int(toff[gi, b, t] + tlen[gi, b, t])
                if hi == lo:
                    continue
                for c in range(lo // P, -(-hi // P)):
                    sl.append((t, c, ct0))
                    ct0 += 1
            segs[gi][b] = sl
    CE = cseq          # total edge chunks per core
    CT = ct0           # total masked meta columns
    NCTMAX = max(
        (len(segs[gi][b]) for gi in range(NG) for b in range(NBANK)),
        default=1,
    )
    NCHMAX = int(nch.max())

    # ---- place edges into slots ----
    order = np.lexsort((src, key_t, dq))  # by (core, g, b, tile), stable
    so = order
    core_o = dq[so]
    g_o = d_g[so]
    b_o = sk[so]
    t_o = d_tile[so]
    # position within (core,g,b,t)
    sk2 = (core_o * NG * NBANK + g_o * NBANK + b_o) * T + t_o
    starts = np.zeros(NC * NKEY + 1, np.int64)
    starts[1:] = np.cumsum(cnt.reshape(-1))
    pos_in = np.arange(len(so)) - starts[sk2]
    slot = toff[g_o, b_o, t_o] + pos_in            # slot within (g,b) stream
    chunk = c0[g_o, b_o] + slot // P               # global chunk col
    part = slot % P

    gidx16 = np.zeros((NC, 16, CE * 8), np.int16)
    gidx16[core_o, part % 16, chunk * 8 + part // 16] = sbr[so].astype(np.int16)
    gidx = np.ascontiguousarray(np.tile(gidx16, (1, 8, 1)))

    # masked per-(chunk,tile) meta columns, packed v = dstl + norm (f32)
    mv_a = np.zeros((NC, P, CT), np.float32)
    # vectorized edge -> ct col: ct = ct_base[g,b,t] + slot//P - clo[g,b,t]
    ct_base = np.zeros((NG, NBANK, T), np.int64)
    clo_arr = np.zeros((NG, NBANK, T), np.int64)
    for gi in range(NG):
        for b in range(NBANK):
            for (t, c, ct) in segs[gi][b]:
                if ct_base[gi, b, t] == 0 and clo_arr[gi, b, t] == 0:
                    ct_base[gi, b, t] = ct
                    clo_arr[gi, b, t] = c
                else:
                    ct_base[gi, b, t] = min(ct_base[gi, b, t], ct)
                    clo_arr[gi, b, t] = min(clo_arr[gi, b, t], c)
    ct_of = (ct_base[g_o, b_o, t_o] + slot // P - clo_arr[g_o, b_o, t_o])
    mv_a[core_o, part, ct_of] = d_loc[so] + norm[so]

    # per-tile self-loop scale dinv^2 (0 on padded rows)
    mself_a = np.zeros((NC, P, T), np.float32)
    for q in range(NC):
        lo = q * cfg.nodes_per_core
        cnt_q = cfg.nodes_per_core
        d2 = dinv[lo: lo + cnt_q] ** 2
        full = np.zeros(cfg.n_loc, np.float32)
        full[:cnt_q] = d2
        mself_a[q] = full.reshape(T, P).T
    mself = mself_a.astype(ml_dtypes.bfloat16)

    # ---- labels: 16 streams by (bank(a), bank(b)) ----
    ela = edge_label_index[0].astype(np.int64)
    elb = edge_label_index[1].astype(np.int64)
    _, _, ak, abr = _place(cfg, ela)
    _, _, bk, bbr = _place(cfg, elb)
    bp_all = ak * NBANK + bk
    lpc = cfg.lab_per_core
    NBP = NBANK * NBANK
    cnts = np.zeros((NC, NBP), np.int64)
    for q in range(NC):
        lo, hi = q * lpc, min((q + 1) * lpc, cfg.n_labels)
        cnts[q] = np.bincount(bp_all[lo:hi], minlength=NBP)
    lkb = (-(-cnts.max(axis=0) // P)).astype(np.int64)
    lchunk0 = np.zeros(NBP + 1, np.int64)
    lchunk0[1:] = np.cumsum(lkb)
    LCp = int(lchunk0[-1])
    LABMAX = int(lkb.max())
    lcol0a = lchunk0[:-1] * 16
    lcol0b = lcol0a + lkb * 8
    LICOLS = LCp * 16

    lidx16 = np.zeros((NC, 16, LICOLS), np.int16)
    order_arr = np.full((NC, LCp * P), -1, np.int64)
    for q in range(NC):
        lo, hi = q * lpc, min((q + 1) * lpc, cfg.n_labels)
        bp_q = bp_all[lo:hi]
        oq = np.argsort(bp_q, kind="stable")
        sbp = bp_q[oq]
        st = np.zeros(NBP + 1, np.int64)
        st[1:] = np.cumsum(cnts[q])
        pos = np.arange(len(oq)) - st[sbp]
        cola = lcol0a[sbp] + pos // 16
        colb = lcol0b[sbp] + pos // 16
        prt = pos % 16
        lidx16[q, prt, cola] = abr[lo:hi][oq].astype(np.int16)
        lidx16[q, prt, colb] = bbr[lo:hi][oq].astype(np.int16)
        sl = (lchunk0[sbp] + pos // P) * P + pos % P
        order_arr[q, sl] = lo + oq
    lidx = np.ascontiguousarray(np.tile(lidx16, (1, 8, 1)))

    # ---- node features shards, feature-major ----
    xT_shards = []
    for q in range(NC):
        lo = q * cfg.nodes_per_core
        xs = np.zeros((cfg.n_loc, P), np.float32)
        xs[: cfg.nodes_per_core] = x[lo: lo + cfg.nodes_per_core]
        xT_shards.append(
            np.ascontiguousarray(xs.T).astype(ml_dtypes.bfloat16)
        )

    iota_rep = np.tile(
        np.arange(P, dtype=np.float32)[None, :], (P, 1)
    ).astype(ml_dtypes.bfloat16)
    iota_col = np.arange(P, dtype=np.float32).reshape(P, 1).astype(
        ml_dtypes.bfloat16
    )

    layout = dict(
        groups=groups, nch=nch, c0=c0, segs=segs, CE=CE, CT=CT,
        NCTMAX=NCTMAX, NCHMAX=NCHMAX,
        lkb=[int(v) for v in lkb], lchunk0=[int(v) for v in lchunk0],
        LCp=LCp, LABMAX=LABMAX,
        lcol0a=[int(v) for v in lcol0a], lcol0b=[int(v) for v in lcol0b],
        LICOLS=LICOLS,
    )
    return dict(gidx=gidx, mv=mv_a, mself=mself, lidx=lidx,
                xT_shards=xT_shards, iota_rep=iota_rep, iota_col=iota_col,
                order_arr=order_arr, layout=layout)


# ------------------------------------------------------------- bass program


def build_program(cfg, lay, linb_sum, phase=99):
    T = cfg.tiles_per_core
    QR = cfg.qrows
    BR = cfg.bank_rows
    QT = cfg.qtiles
    groups = lay["groups"]
    NG = len(groups)
    nch, c0g, segs = lay["nch"], lay["c0"], lay["segs"]
    CE, CT, NCTMAX, NCHMAX = lay["CE"], lay["CT"], lay["NCTMAX"], lay["NCHMAX"]
    LCp, LABMAX = lay["LCp"], lay["LABMAX"]
    lkb, lchunk0 = lay["lkb"], lay["lchunk0"]
    lcol0a, lcol0b = lay["lcol0a"], lay["lcol0b"]
    rg = [list(range(NC))]

    nc = bacc.Bacc(None, target_bir_lowering=False, debug=False,
                   dynamic_dma_scratch_size=16384, num_swdge_queues=4)
    qrr = [0]

    def next_q():
        qrr[0] = (qrr[0] + 1) % 4
        return qrr[0]

    xT_d = nc.declare_dram_parameter("xTq", [P, cfg.n_loc], BF, False)
    gidx_d = nc.declare_dram_parameter("gidx", [P, CE * 8], I16, False)
    mv_d = nc.declare_dram_parameter("mv", [P, CT], F32, False)
    mself_d = nc.declare_dram_parameter("mself", [P, T], BF, False)
    iota_d = nc.declare_dram_parameter("iota", [P, P], BF, False)
    iotac_d = nc.declare_dram_parameter("iotac", [P, 1], BF, False)
    lidx_d = nc.declare_dram_parameter("lidx", [P, lay["LICOLS"]], I16, False)
    w1_d = nc.declare_dram_parameter("w1", [P, P], BF, False)
    w2_d = nc.declare_dram_parameter("w2", [P, P], BF, False)
    b1c_d = nc.declare_dram_parameter("b1c", [P, 1], F32, False)
    b2_d = nc.declare_dram_parameter("b2bc", [P, P], F32, False)
    wv_d = nc.declare_dram_parameter("wvrep", [P, LABMAX * P], F32, False)
    res_d = nc.declare_dram_parameter("res", [P, LCp], F32, True)

    h1_sh = [nc.dram_tensor(f"h1sh{k}", [QR, P], BF) for k in range(4)]
    h2_sh = [nc.dram_tensor(f"h2sh{k}", [QR, P], BF) for k in range(4)]
    o2_sh = [nc.dram_tensor(f"o2sh{k}", [QR, P], BF) for k in range(4)]
    htab1 = [
        nc.dram_tensor(f"htab1_{k}", [BR, P], BF, addr_space="Shared")
        for k in range(4)
    ]
    htab2 = [
        nc.dram_tensor(f"htab2_{k}", [BR, P], BF, addr_space="Shared")
        for k in range(4)
    ]
    o2tab = [
        nc.dram_tensor(f"o2tab_{k}", [BR, P], BF, addr_space="Shared")
        for k in range(4)
    ]

    AF = mybir.ActivationFunctionType
    OP = mybir.AluOpType

    with TileContext(nc) as tc:
        with (
            tc.tile_pool(name="const", bufs=1) as cp,
            tc.tile_pool(name="xload", bufs=2) as xp,
            tc.tile_pool(name="gemmev", bufs=3) as gep,
            tc.tile_pool(name="gitile", bufs=3) as gip,
            tc.tile_pool(name="hgb", bufs=10) as hp,
            tc.tile_pool(name="wgb", bufs=4) as wp,
            tc.tile_pool(name="aggev", bufs=3) as aep,
            tc.tile_pool(name="lab", bufs=2) as lp,
            tc.tile_pool(name="ps_gemm", bufs=2, space="PSUM") as psg,
            tc.tile_pool(name="ps_agg", bufs=3, space="PSUM") as psa,
        ):
            nc.gpsimd.load_library(mlp)
            # ---- persistent SBUF ----
            mv_sb = cp.tile([P, CT], F32)
            nc.sync.dma_start(out=mv_sb[:], in_=mv_d[:, :])
            mself_sb = cp.tile([P, T], BF)
            nc.sync.dma_start(out=mself_sb[:], in_=mself_d[:, :])
            iota_sb = cp.tile([P, P], BF)
            nc.sync.dma_start(out=iota_sb[:], in_=iota_d[:, :])
            iotac_sb = cp.tile([P, 1], BF)
            nc.sync.dma_start(out=iotac_sb[:], in_=iotac_d[:, :])
            lidx_sb = cp.tile([P, lay["LICOLS"]], I16)
            nc.sync.dma_start(out=lidx_sb[:], in_=lidx_d[:, :])
            w1_sb = cp.tile([P, P], BF)
            nc.sync.dma_start(out=w1_sb[:], in_=w1_d[:, :])
            w2_sb = cp.tile([P, P], BF)
            nc.sync.dma_start(out=w2_sb[:], in_=w2_d[:, :])
            b1c_sb = cp.tile([P, 1], F32)
            nc.sync.dma_start(out=b1c_sb[:], in_=b1c_d[:, :])
            b2_sb = cp.tile([P, P], F32)
            nc.sync.dma_start(out=b2_sb[:], in_=b2_d[:, :])
            wv_sb = cp.tile([P, LABMAX * P], F32)
            nc.sync.dma_start(out=wv_sb[:], in_=wv_d[:, :])
            res_sb = cp.tile([P, LCp], F32)

            selfh1 = cp.tile([P, T * P], BF)   # dinv^2-scaled own h1 tiles
            selfh2 = cp.tile([P, T * P], BF)
            # identity one-hot for self chunks
            idmat = cp.tile([P, P], BF)
            nc.vector.tensor_tensor(
                out=idmat[:],
                in0=iota_sb[:, :P],
                in1=iotac_sb[:].to_broadcast([P, P]),
                op=OP.is_equal,
            )

            # ---- sharded GEMM1 + quarter AllGathers ----
            def gemm1():
                for k in range(4):
                    lhsT = xp.tile([P, QT * P], BF, tag="x")
                    nc.scalar.dma_start(
                        out=lhsT[:],
                        in_=xT_d[:, k * QT * P: (k + 1) * QT * P],
                    )
                    G1 = 4
                    for t0 in range(k * QT, (k + 1) * QT, G1):
                        gs = min(G1, (k + 1) * QT - t0)
                        x0 = (t0 - k * QT) * P
                        pg = psg.tile([P, 4 * P], F32, tag="g1")
                        for i in range(gs):
                            nc.tensor.matmul(
                                out=pg[:, i * P: (i + 1) * P],
                                lhsT=lhsT[:, x0 + i * P: x0 + (i + 1) * P],
                                rhs=w1_sb[:],
                                start=True, stop=True,
                            )
                        hb = gep.tile([P, 4 * P], BF, tag="hb")
                        nc.scalar.activation(
                            hb[:, : gs * P], pg[:, : gs * P], AF.Copy
                        )
                        # self-scaled copy (dinv^2 per node row)
                        nc.vector.tensor_tensor(
                            out=selfh1[:, t0 * P: (t0 + gs) * P]
                            .rearrange("p (g e) -> p g e", e=P),
                            in0=hb[:, : gs * P]
                            .rearrange("p (g e) -> p g e", e=P),
                            in1=mself_sb[:, t0: t0 + gs]
                            .to_broadcast([P, gs, P]),
                            op=OP.mult,
                        )
                        nc.sync.dma_start(
                            out=h1_sh[k][
                                (t0 - k * QT) * P: (t0 - k * QT + gs) * P, :
                            ].rearrange("(i p) j -> p i j", p=P),
                            in_=hb[:, : gs * P]
                            .rearrange("p (i j) -> p i j", j=P),
                        )
                    nc.gpsimd.collective_compute(
                        "AllGather", OP.bypass, replica_groups=rg,
                        ins=[h1_sh[k][:, :]], outs=[htab1[k][:, :]],
                    )

            # ---- label stream emission (split per gather side) ----
            lab_tiles = {}

            def emit_label_gather(bp, side):
                nchb = lkb[bp]
                if nchb == 0:
                    return
                b0, b1 = divmod(bp, NBANK)
                tile_ = lp.tile([P, LABMAX * P], BF, tag=f"l{side}")
                lab_tiles[(bp, side)] = tile_
                bank = b0 if side == 0 else b1
                col0 = lcol0a[bp] if side == 0 else lcol0b[bp]
                for cc in range(0, nchb, MAXCH):
                    cw = min(MAXCH, nchb - cc)
                    nc.gpsimd.dma_gather(
                        tile_[:, cc * P: (cc + cw) * P].rearrange(
                            "p (c e) -> p c e", e=P
                        ),
                        o2tab[bank][:, :],
                        lidx_sb[:, col0 + cc * 8: col0 + (cc + cw) * 8],
                        cw * P, cw * P, P,
                        queue_num=next_q(),
                    )

            def emit_label_reduce(bp):
                nchb = lkb[bp]
                if nchb == 0:
                    return
                a = lab_tiles.pop((bp, 0))
                bb = lab_tiles.pop((bp, 1))
                prod = lp.tile([P, LABMAX * P], F32, tag="prod")
                nc.vector.tensor_tensor(
                    out=prod[:, : nchb * P],
                    in0=a[:, : nchb * P],
                    in1=bb[:, : nchb * P],
                    op=OP.mult,
                )
                nc.vector.tensor_tensor(
                    out=prod[:, : nchb * P],
                    in0=prod[:, : nchb * P],
                    in1=wv_sb[:, : nchb * P],
                    op=OP.mult,
                )
                nc.vector.reduce_sum(
                    res_sb[:, lchunk0[bp]: lchunk0[bp] + nchb],
                    prod[:, : nchb * P].rearrange("p (g e) -> p g e", e=P),
                    axis=mybir.AxisListType.X,
                )

            # ---- aggregation layer (fused gemm2 after layer 1) ----
            def emit_group_compute(layer, gi, t0, gs, htiles, sh):
                # W tiles for the group (one fused DVE op per bank)
                wtiles = {}
                ct_firsts = {}
                for b in range(NBANK):
                    sl = segs[gi][b]
                    if not sl:
                        continue
                    nct = len(sl)
                    ct_first = sl[0][2]
                    ct_firsts[b] = ct_first
                    w = wp.tile([P, NCTMAX * P], BF, tag="w")
                    wtiles[b] = w
                    w3 = w[:, : nct * P].rearrange("p (g e) -> p g e", e=P)
                    nc.vector._custom_dve(
                        W_ONEHOT,
                        out=w3,
                        in0=mv_sb[:, ct_first: ct_first + nct]
                        .to_broadcast([P, nct, P]),
                        s0=0.0, s1=float(P),
                    )
                # per-tile segment lists: a psum region accumulation must
                # run start->stop contiguously within a bank
                tsegs = {t: [] for t in range(t0, t0 + gs)}
                for b in range(NBANK):
                    if segs[gi][b]:
                        for (t, c, ct) in segs[gi][b]:
                            tsegs[t].append((b, c, ct))
                pgrp = psa.tile([P, GMAX * P], F32)
                pts = {}
                for i, t in enumerate(range(t0, t0 + gs)):
                    pts[t] = pgrp[:, i * P: (i + 1) * P]
                    shs = sh[:, t * P: (t + 1) * P]
                    onlyself = not tsegs[t]
                    if layer == 1:
                        nc.tensor.matmul(
                            out=pts[t], lhsT=shs, rhs=idmat[:],
                            start=True, stop=onlyself,
                        )
                    else:
                        nc.tensor.matmul(
                            out=pts[t], lhsT=idmat[:], rhs=shs,
                            start=True, stop=onlyself,
                        )
                    nseg = len(tsegs[t])
                    for si, (b, c, ct) in enumerate(tsegs[t]):
                        hs = htiles[(gi, b)][:, c * P: (c + 1) * P]
                        ws = wtiles[b][
                            :, (ct - ct_firsts[b]) * P:
                            (ct - ct_firsts[b] + 1) * P]
                        stop = si == nseg - 1
                        if layer == 1:
                            nc.tensor.matmul(
                                out=pts[t], lhsT=hs, rhs=ws,
                                start=False, stop=stop,
                            )
                        else:
                            nc.tensor.matmul(
                                out=pts[t], lhsT=ws, rhs=hs,
                                start=False, stop=stop,
                            )
                # evictions
                k = t0 // QT
                tq0 = t0 - k * QT
                if layer == 1:
                    ob = aep.tile([P, GMAX * P], BF, tag="ob")
                    h2b = aep.tile([P, GMAX * P], BF, tag="h2b")
                    for i, t in enumerate(range(t0, t0 + gs)):
                        nc.scalar.activation(
                            ob[:, i * P: (i + 1) * P], pts[t],
                            AF.Relu, bias=b1c_sb[:],
                        )
                    for i, t in enumerate(range(t0, t0 + gs)):
                        pg2 = psg.tile([P, P], F32, tag="g2")
                        nc.tensor.matmul(
                            out=pg2[:],
                            lhsT=ob[:, i * P: (i + 1) * P],
                            rhs=w2_sb[:],
                            start=True, stop=True,
                        )
                        nc.scalar.activation(
                            h2b[:, i * P: (i + 1) * P], pg2[:], AF.Copy
                        )
                    nc.vector.tensor_tensor(
                        out=selfh2[:, t0 * P: (t0 + gs) * P]
                        .rearrange("p (g e) -> p g e", e=P),
                        in0=h2b[:, : gs * P]
                        .rearrange("p (g e) -> p g e", e=P),
                        in1=mself_sb[:, t0: t0 + gs]
                        .to_broadcast([P, gs, P]),
                        op=OP.mult,
                    )
                    nc.sync.dma_start(
                        out=h2_sh[k][tq0 * P: (tq0 + gs) * P, :]
                        .rearrange("(i p) j -> p i j", p=P),
                        in_=h2b[:, : gs * P]
                        .rearrange("p (i j) -> p i j", j=P),
                    )
                else:
                    o2b = aep.tile([P, GMAX * P], BF, tag="o2b")
                    for i, t in enumerate(range(t0, t0 + gs)):
                        t1 = aep.tile([P, P], F32, tag="t1")
                        nc.vector.tensor_tensor(
                            out=t1[:], in0=pts[t], in1=b2_sb[:],
                            op=OP.add,
                        )
                        nc.scalar.activation(
                            o2b[:, i * P: (i + 1) * P], t1[:], AF.Relu
                        )
                    nc.sync.dma_start(
                        out=o2_sh[k][tq0 * P: (tq0 + gs) * P, :]
                        .rearrange("(i p) j -> p i j", p=P),
                        in_=o2b[:, : gs * P]
                        .rearrange("p (i j) -> p i j", j=P),
                    )

            def agg(layer, with_labels=False):
                htab = htab1 if layer == 1 else htab2
                sh = selfh1 if layer == 1 else selfh2
                qorder = [0, 1, 2, 3] if layer == 1 else [3, 0, 1, 2]
                NBP = NBANK * NBANK
                lab_gathered = set()
                lab_reduced = set()

                def emit_eligible_labels(done_banks):
                    # whole streams only: a held single-side tile would
                    # WAR-deadlock the small label pool
                    for bp in range(NBP):
                        if lkb[bp] == 0 or bp in lab_reduced:
                            continue
                        b0, b1 = divmod(bp, NBANK)
                        if b0 in done_banks and b1 in done_banks:
                            lab_reduced.add(bp)
                            emit_label_gather(bp, 0)
                            emit_label_gather(bp, 1)
                            emit_label_reduce(bp)

                done_banks = set()
                for qpos, k in enumerate(qorder):
                    gl = [gi for gi, (t0, gs) in enumerate(groups)
                          if t0 // QT == k]
                    # pairs of groups: gathers bank-interleaved, then compute
                    pi = 0
                    while pi < len(gl):
                        pair = gl[pi: pi + 2]
                        pi += 2
                        gts = {}
                        for gi in pair:
                            chg = int(nch[gi].sum())
                            if chg:
                                gt = gip.tile(
                                    [P, NCHMAX * NBANK * 8], I16, tag="gi"
                                )
                                gts[gi] = gt
                                nc.sync.dma_start(
                                    out=gt[:, : chg * 8],
                                    in_=gidx_d[
                                        :, c0g[gi, 0] * 8:
                                        (c0g[gi, 0] + chg) * 8
                                    ],
                                )
                        htiles = {}
                        for b in range(NBANK):
                            for gi in pair:
                                ch = int(nch[gi, b])
                                if ch == 0:
                                    continue
                                h = hp.tile([P, NCHMAX * P], BF, tag="h")
                                htiles[(gi, b)] = h
                                gtoff = (c0g[gi, b] - c0g[gi, 0]) * 8
                                for cc in range(0, ch, MAXCH):
                                    cw = min(MAXCH, ch - cc)
                                    nc.gpsimd.dma_gather(
                                        h[:, cc * P: (cc + cw) * P]
                                        .rearrange("p (c e) -> p c e", e=P),
                                        htab[b][:, :],
                                        gts[gi][
                                            :, gtoff + cc * 8:
                                            gtoff + (cc + cw) * 8
                                        ],
                                        cw * P, cw * P, P,
                                        queue_num=next_q(),
                                    )
                        for gi in pair:
                            t0, gs = groups[gi]
                            emit_group_compute(layer, gi, t0, gs, htiles, sh)
                    # quarter complete -> collective
                    if layer == 1:
                        nc.gpsimd.collective_compute(
                            "AllGather", OP.bypass, replica_groups=rg,
                            ins=[h2_sh[k][:, :]], outs=[htab2[k][:, :]],
                        )
                    else:
                        nc.gpsimd.collective_compute(
                            "AllGather", OP.bypass, replica_groups=rg,
                            ins=[o2_sh[k][:, :]], outs=[o2tab[k][:, :]],
                        )
                        # labels from quarters produced BEFORE this one
                        # (their collectives have had a quarter to land)
                        if with_labels and qpos >= 1:
                            emit_eligible_labels(set(qorder[:qpos]))
                    done_banks.add(k)
                if layer == 2 and with_labels:
                    emit_eligible_labels(done_banks)
                    nc.vector.tensor_scalar_add(
                        res_sb[:], res_sb[:], float(linb_sum)
                    )
                    nc.sync.dma_start(out=res_d[:, :], in_=res_sb[:])

            def probe(src_ap, cast=True):
                prb = cp.tile([P, P], F32)
                if cast:
                    tmp = cp.tile([P, P], BF)
                    nc.sync.dma_start(out=tmp[:], in_=src_ap)
                    nc.vector.tensor_copy(prb[:], tmp[:])
                pb = min(LCp, P)
                nc.sync.dma_start(out=res_d[:, :pb], in_=prb[:, :pb])

            if phase >= 2:
                gemm1()
            if phase == 2:
                probe(htab1[0][0:P, :])
            if phase >= 3:
                agg(1)
            if phase == 3:
                probe(htab2[0][0:P, :])
            if phase >= 4:
                agg(2, with_labels=(phase >= 5))
            if phase == 4:
                probe(o2tab[0][0:P, :])

    nc.finalize()
    return nc


# ------------------------------------------------------------------ driver


def make_in_maps(cfg, prep, W1, b1, W2, b2, lin_W, lin_b):
    wv = lin_W.astype(np.float32).sum(axis=1)
    lay = prep["layout"]
    consts = dict(
        iota=prep["iota_rep"],
        iotac=prep["iota_col"],
        w1=W1.astype(np.float32).astype(ml_dtypes.bfloat16),
        w2=W2.astype(np.float32).astype(ml_dtypes.bfloat16),
        b1c=b1.astype(np.float32).reshape(P, 1),
        b2bc=np.tile(b2.astype(np.float32)[None, :], (P, 1)),
        wvrep=np.tile(wv[None, :], (P, lay["LABMAX"])),
    )
    in_maps = []
    for q in range(NC):
        m = dict(consts)
        m.update(
            xTq=prep["xT_shards"][q],
            gidx=prep["gidx"][q],
            mv=prep["mv"][q],
            mself=prep["mself"][q],
            lidx=prep["lidx"][q],
        )
        in_maps.append(m)
    return in_maps


def assemble_output(cfg, prep, results):
    out = np.zeros(cfg.n_labels, np.float32)
    order_arr = prep["order_arr"]
    for q in range(NC):
        r = np.asarray(results[q]["res"], np.float32)
        v = r.T.reshape(-1)
        m = order_arr[q] >= 0
        out[order_arr[q][m]] = v[m]
    return out


def run(cfg, x, edge_index, edge_weight, edge_label_index,
        W1, b1, W2, b2, lin_W, lin_b, trace=False, phase=99):
    global LAST_EXEC_NS, LAST_RESULTS
    prep = preprocess(cfg, np.asarray(x), np.asarray(edge_index),
                      np.asarray(edge_weight), np.asarray(edge_label_index))
    linb_sum = float(np.asarray(lin_b, np.float64).sum())
    nc = build_program(cfg, prep["layout"], linb_sum, phase=phase)
    in_maps = make_in_maps(cfg, prep, W1, b1, W2, b2, lin_W, lin_b)
    res = run_bass_kernel_spmd(
        nc, in_maps, list(range(NC)), trace=trace
    )
    LAST_EXEC_NS = res.exec_time_ns
    LAST_RESULTS = res
    return assemble_output(cfg, prep, res.results)


def kernel(x, edge_index, edge_weight, edge_label_index,
           W1, b1, W2, b2, lin_W, lin_b):
    trace = bool(os.environ.get("KERNEL_TRACE"))
    return run(FULL, x, edge_index, edge_weight, edge_label_index,
               W1, b1, W2, b2, lin_W, lin_b, trace=trace)
